# revision 1
# baseline (speedup 1.0000x reference)
"""Self-contained 8-core Trainium2 kernel for the 6-layer dense transformer.

Sharding: token-parallel. Core c owns batch b=c//2, sequence half h=c%2
(512 tokens). Per layer, each core computes K/V for its own tokens and
AllGathers them; causal attention then runs locally over the (batch's) full
key prefix, selected from the AG buffer with host-computed indirect-DMA
indices. The LM head is vocab-sharded: after a final AllGather of the
normalized activations, each core computes logits for all 4096 tokens over
its 4000-column vocab slice.

Activations live feature-major (x^T: [D, tokens]) so every matmul contracts
over the partition axis without transposes. Matmuls run in float32r
(TF32-like, ~1e-4 relative error, full PE rate); LayerNorm statistics use
matmul-with-ones partition reductions; softmax runs on transposed scores
with the denominator produced by a ones-column appended to V.
"""

import numpy as np

B, T, D, H, HS, L, DFF, V = 4, 1024, 1024, 16, 64, 6, 4096, 32000
NCORES = 8
TC = 512            # tokens per core
P = 128
VS = V // NCORES    # 4000 vocab cols per core
EPS = 1e-5
NEG = -30000.0
QS = [0, 128, 256, 256]   # q-slice starts for key-chunks 4..7 (capped at 256)

_CACHE = {}
TRACE = False
LAST_RESULTS = None
SKIP_COLL = False   # timing variant: replace AllGathers with local 2MB DMA copies
SKIP_LM = False     # timing variant: skip the LM head


def _build():
    import concourse.bacc as bacc
    import concourse.tile as tile
    import concourse.mybir as mybir
    import concourse.bass as bass
    from concourse.masks import make_identity
    from contextlib import ExitStack

    f32 = mybir.dt.float32
    f32r = mybir.dt.float32r
    bf16 = mybir.dt.bfloat16
    i32 = mybir.dt.int32
    AF = mybir.ActivationFunctionType
    ALU = mybir.AluOpType

    nc = bacc.Bacc(None, target_bir_lowering=False, debug=False,
                   num_devices=NCORES)

    # ---- parameters ----
    tokidx = nc.declare_dram_parameter("tokidx", [TC, 1], i32, isOutput=False)
    pos = nc.declare_dram_parameter("pos", [TC, D], f32, isOutput=False)
    tokemb = nc.declare_dram_parameter("tokemb", [V, D], f32, isOutput=False)
    wqT = nc.declare_dram_parameter("wqT", [L, D, D], f32r, isOutput=False)
    wkT = nc.declare_dram_parameter("wkT", [L, D, D], f32r, isOutput=False)
    wvT = nc.declare_dram_parameter("wvT", [L, D, D], f32r, isOutput=False)
    wo = nc.declare_dram_parameter("wo", [L, D, D], f32r, isOutput=False)
    w1 = nc.declare_dram_parameter("w1", [L, D, DFF], f32r, isOutput=False)
    w2 = nc.declare_dram_parameter("w2", [L, DFF, D], f32r, isOutput=False)
    wout = nc.declare_dram_parameter("wout", [D, VS], f32r, isOutput=False)
    ln1g = nc.declare_dram_parameter("ln1g", [L, D], f32, isOutput=False)
    ln1b = nc.declare_dram_parameter("ln1b", [L, D], f32, isOutput=False)
    ln2g = nc.declare_dram_parameter("ln2g", [L, D], f32, isOutput=False)
    ln2b = nc.declare_dram_parameter("ln2b", [L, D], f32, isOutput=False)
    lnfg = nc.declare_dram_parameter("lnfg", [1, D], f32, isOutput=False)
    lnfb = nc.declare_dram_parameter("lnfb", [1, D], f32, isOutput=False)
    bo_p = nc.declare_dram_parameter("bo", [L, D], f32, isOutput=False)
    b1_p = nc.declare_dram_parameter("b1", [L, DFF], f32, isOutput=False)
    b2_p = nc.declare_dram_parameter("b2", [L, D], f32, isOutput=False)
    bout = nc.declare_dram_parameter("bout", [1, VS], f32r, isOutput=False)
    mask0 = nc.declare_dram_parameter("mask0", [4, P, TC], bf16, isOutput=False)
    mask1 = nc.declare_dram_parameter("mask1", [4, P, TC], bf16, isOutput=False)
    kidx = nc.declare_dram_parameter("kidx", [P, H], i32, isOutput=False)
    vidx = nc.declare_dram_parameter("vidx", [P, 32], i32, isOutput=False)
    out = nc.declare_dram_parameter("out", [B * T, VS], f32, isOutput=True)

    RG = [list(range(NCORES))]

    with tile.TileContext(nc) as tc:
        outer = ExitStack()
        singles = outer.enter_context(tc.tile_pool(name="singles", bufs=1))
        dramp = outer.enter_context(tc.tile_pool(name="dramp", bufs=1, space="DRAM"))

        # ---- internal DRAM ----
        k_loc = dramp.tile([D, TC], f32r, name="k_loc")
        v_loc = dramp.tile([4 * TC, 260], f32r, name="v_loc")
        xf_loc = dramp.tile([D, TC], f32r, name="xf_loc")
        k_ags = [dramp.tile([NCORES * D, TC], f32r, name=f"k_ag_{i}",
                            addr_space="Shared") for i in range(L)]
        v_ags = [dramp.tile([NCORES * 4 * TC, 260], f32r, name=f"v_ag_{i}",
                            addr_space="Shared") for i in range(L)]
        xf_ag = dramp.tile([NCORES * D, TC], f32r, name="xf_ag", addr_space="Shared")

        # constants
        ones_f = singles.tile([P, 144], f32, name="ones_f")
        nc.vector.memset(ones_f[:], 1.0)
        ones_r = singles.tile([P, 144], f32r, name="ones_r")
        nc.vector.tensor_copy(out=ones_r[:], in_=ones_f[:])
        eps_t = singles.tile([1, 1], f32, name="eps_t")
        nc.vector.memset(eps_t[:], EPS)
        ident = singles.tile([P, P], f32, name="ident")
        make_identity(nc, ident[:])
        kidx_t = singles.tile([P, H], i32, name="kidx_t")
        nc.sync.dma_start(out=kidx_t[:], in_=kidx[:])
        vidx_t = singles.tile([P, 32], i32, name="vidx_t")
        nc.sync.dma_start(out=vidx_t[:], in_=vidx[:])
        m0_t = []
        m1_t = []
        for j in range(4):
            mt = singles.tile([P, TC], bf16, name=f"m0_{j}")
            nc.sync.dma_start(out=mt[:], in_=mask0[j])
            m0_t.append(mt)
            mt = singles.tile([P, TC], bf16, name=f"m1_{j}")
            nc.sync.dma_start(out=mt[:], in_=mask1[j])
            m1_t.append(mt)

        est = ExitStack()
        lp = est.enter_context(tc.tile_pool(name="lp", bufs=1))      # xr/xln
        big = est.enter_context(tc.tile_pool(name="big", bufs=1))    # 16 shared slots
        wA = est.enter_context(tc.tile_pool(name="wA", bufs=2))      # [P,8,128] weights
        wB = est.enter_context(tc.tile_pool(name="wB", bufs=2))      # [P,8,256] wv quarters
        sp = est.enter_context(tc.tile_pool(name="sp", bufs=2))      # stream tiles
        kvp = est.enter_context(tc.tile_pool(name="kvp", bufs=2))    # kv copyback
        ktp = est.enter_context(tc.tile_pool(name="ktp", bufs=2))    # K gathers
        esp = est.enter_context(tc.tile_pool(name="esp", bufs=3))    # exp(scores)
        vtp = est.enter_context(tc.tile_pool(name="vtp", bufs=1))    # V gathers (8 tags)
        otp = est.enter_context(tc.tile_pool(name="otp", bufs=2))    # o tmp / recip
        stp = est.enter_context(tc.tile_pool(name="stp", bufs=1))    # LN stats [1,*]

        ps_mm = est.enter_context(tc.tile_pool(name="ps_mm", bufs=2, space="PSUM"))
        ps_o = est.enter_context(tc.tile_pool(name="ps_o", bufs=2, space="PSUM"))
        ps_st = est.enter_context(tc.tile_pool(name="ps_st", bufs=1, space="PSUM"))
        ps_bc = est.enter_context(tc.tile_pool(name="ps_bc", bufs=1, space="PSUM"))

        def mmtile():
            return ps_mm.tile([P, TC], f32, name="mm", tag="mm")

        xr = [lp.tile([P, TC], f32, name=f"xr_{j}", tag=f"xr_{j}") for j in range(8)]

        def xln_tiles():
            return [lp.tile([P, TC], f32r, name=f"xln_{j}", tag=f"xln_{j}")
                    for j in range(8)]

        def big_tile(i, name, dtype=f32r):
            return big.tile([P, TC], dtype, name=name, tag=f"big_{i}")

        # ---- embedding: gather + pos add + transpose into xr ----
        with tc.tile_pool(name="embp", bufs=1) as embp:
            for t4 in range(4):
                it = embp.tile([P, 1], i32, name="emb_idx", tag="emb_idx")
                nc.sync.dma_start(out=it[:], in_=tokidx[t4 * P:(t4 + 1) * P, :])
                gx = embp.tile([P, D], f32, name="emb_gx", tag="emb_gx")
                nc.gpsimd.indirect_dma_start(
                    out=gx[:], out_offset=None, in_=tokemb[:],
                    in_offset=bass.IndirectOffsetOnAxis(ap=it[:, :1], axis=0))
                pt = embp.tile([P, D], f32, name="emb_pos", tag="emb_pos")
                nc.sync.dma_start(out=pt[:], in_=pos[t4 * P:(t4 + 1) * P, :])
                xs = embp.tile([P, D], f32, name="emb_xs", tag="emb_xs")
                nc.vector.tensor_add(out=xs[:], in0=gx[:], in1=pt[:])
                for j in range(8):
                    tp = mmtile()
                    nc.tensor.transpose(out=tp[:, 0:P], in_=xs[:, j * P:(j + 1) * P],
                                        identity=ident[:])
                    nc.scalar.activation(out=xr[j][:, t4 * P:(t4 + 1) * P],
                                         in_=tp[:, 0:P], func=AF.Copy)

        # ---- ones columns of v_loc (V writes never touch them) ----
        vl_view = v_loc[:].rearrange("(hq t) (h c) -> hq t h c", hq=4, c=65)
        for hq in range(4):
            for t4 in range(4):
                nc.sync.dma_start(
                    out=vl_view[hq, t4 * P:(t4 + 1) * P, 0:4, 64:65],
                    in_=ones_r[:, 128:132])

        def layer_norm(g_t, gcol, b_t, bcol, out_tiles):
            """xr (f32) -> out_tiles (f32r); feature-major LN over partitions."""
            sum_ps = ps_st.tile([1, TC], f32, name="sum_ps", tag="st_a")
            sumsq_ps = ps_st.tile([1, TC], f32, name="sumsq_ps", tag="st_b")
            for j in range(8):
                xc = sp.tile([P, TC], f32r, name="ln_xc", tag="ln_xc")
                nc.scalar.activation(out=xc[:], in_=xr[j][:], func=AF.Copy)
                sq = sp.tile([P, TC], f32r, name="ln_sq", tag="ln_sq")
                nc.scalar.activation(out=sq[:], in_=xr[j][:], func=AF.Square)
                nc.tensor.matmul(out=sum_ps[:], lhsT=ones_r[:, 0:1], rhs=xc[:],
                                 start=(j == 0), stop=(j == 7))
                nc.tensor.matmul(out=sumsq_ps[:], lhsT=ones_r[:, 1:2], rhs=sq[:],
                                 start=(j == 0), stop=(j == 7))
            nmean = stp.tile([1, TC], f32, name="ln_nmean", tag="ln_nmean")
            nc.scalar.activation(out=nmean[:], in_=sum_ps[:], func=AF.Copy,
                                 scale=-1.0 / D)
            ms = stp.tile([1, TC], f32, name="ln_ms", tag="ln_ms")
            nc.scalar.activation(out=ms[:], in_=sumsq_ps[:], func=AF.Copy,
                                 scale=1.0 / D)
            m2 = stp.tile([1, TC], f32, name="ln_m2", tag="ln_m2")
            nc.vector.tensor_mul(out=m2[:], in0=nmean[:], in1=nmean[:])
            var = stp.tile([1, TC], f32, name="ln_var", tag="ln_var")
            nc.vector.tensor_tensor(out=var[:], in0=ms[:], in1=m2[:],
                                    op=ALU.subtract)
            std = stp.tile([1, TC], f32, name="ln_std", tag="ln_std")
            nc.scalar.activation(out=std[:], in_=var[:], func=AF.Sqrt,
                                 bias=eps_t[:], scale=1.0)
            rc = stp.tile([1, 2 * TC], f32r, name="ln_rc", tag="ln_rc")
            with nc.allow_low_precision(reason="f32r rounding of rstd intended"):
                nc.vector.reciprocal(out=rc[:, 0:TC], in_=std[:])
            nc.vector.tensor_mul(out=rc[:, TC:2 * TC], in0=nmean[:], in1=rc[:, 0:TC])
            bc_a = ps_bc.tile([P, TC], f32, name="bc_a", tag="bc_a")
            nc.tensor.matmul(out=bc_a[:], lhsT=ones_r[0:1, 0:P], rhs=rc[:, 0:TC],
                             start=True, stop=True)
            bc_c = ps_bc.tile([P, TC], f32, name="bc_c", tag="bc_c")
            nc.tensor.matmul(out=bc_c[:], lhsT=ones_r[0:1, 0:P],
                             rhs=rc[:, TC:2 * TC], start=True, stop=True)
            for j in range(8):
                t1 = sp.tile([P, TC], f32, name="ln_t1", tag="ln_t1")
                nc.vector.tensor_mul(out=t1[:], in0=xr[j][:], in1=bc_a[:])
                nc.vector.tensor_add(out=t1[:], in0=t1[:], in1=bc_c[:])
                nc.vector.tensor_scalar(
                    out=out_tiles[j][:], in0=t1[:],
                    scalar1=g_t[:, gcol + j:gcol + j + 1],
                    scalar2=b_t[:, bcol + j:bcol + j + 1],
                    op0=ALU.mult, op1=ALU.add)

        # ================= layers =================
        for l in range(L):
            lnp = sp.tile([P, 48], f32, name="lnp", tag="lnp")
            nc.sync.dma_start(out=lnp[:, 0:8], in_=ln1g[l].rearrange("(j p) -> p j", p=P))
            nc.sync.dma_start(out=lnp[:, 8:16], in_=ln1b[l].rearrange("(j p) -> p j", p=P))
            nc.sync.dma_start(out=lnp[:, 16:24], in_=ln2g[l].rearrange("(j p) -> p j", p=P))
            nc.sync.dma_start(out=lnp[:, 24:32], in_=ln2b[l].rearrange("(j p) -> p j", p=P))
            nc.sync.dma_start(out=lnp[:, 32:40], in_=bo_p[l].rearrange("(j p) -> p j", p=P))
            nc.sync.dma_start(out=lnp[:, 40:48], in_=b2_p[l].rearrange("(j p) -> p j", p=P))
            b1f_t = sp.tile([P, 32], f32, name="b1f_t", tag="b1f_t")
            nc.sync.dma_start(out=b1f_t[:], in_=b1_p[l].rearrange("(j p) -> p j", p=P))

            xln = xln_tiles()
            layer_norm(lnp, 0, lnp, 8, xln)

            # ---- K projection -> k_loc ----
            wk_l = wkT[l].rearrange("(k p) n -> p k n", p=P)
            for oc in range(8):
                wt = wA.tile([P, 8, P], f32r, name="wk_t", tag="wA")
                nc.sync.dma_start(out=wt[:], in_=wk_l[:, :, oc * P:(oc + 1) * P])
                ps = mmtile()
                for k in range(8):
                    nc.tensor.matmul(out=ps[:], lhsT=wt[:, k, :], rhs=xln[k][:],
                                     start=(k == 0), stop=(k == 7))
                ksb = kvp.tile([P, TC], f32r, name="ksb", tag="ksb")
                nc.scalar.activation(out=ksb[:], in_=ps[:], func=AF.Copy)
                nc.sync.dma_start(out=k_loc[oc * P:(oc + 1) * P, :], in_=ksb[:])

            # ---- V projection -> v_loc (head-quarter rows, 65-strided) ----
            wv_l = wvT[l].rearrange("(k p) n -> p k n", p=P)
            for hq in range(4):
                wt = wB.tile([P, 8, 256], f32r, name="wv_t", tag="wB")
                nc.sync.dma_start(out=wt[:], in_=wv_l[:, :, hq * 256:(hq + 1) * 256])
                for t4 in range(4):
                    ps = mmtile()
                    for k in range(8):
                        nc.tensor.matmul(out=ps[:, 0:256],
                                         lhsT=xln[k][:, t4 * P:(t4 + 1) * P],
                                         rhs=wt[:, k, :],
                                         start=(k == 0), stop=(k == 7))
                    vsb = kvp.tile([P, 256], f32r, name="vsb", tag="vsb")
                    nc.scalar.activation(out=vsb[:], in_=ps[:, 0:256], func=AF.Copy)
                    nc.sync.dma_start(
                        out=vl_view[hq, t4 * P:(t4 + 1) * P, 0:4, 0:64],
                        in_=vsb[:].rearrange("p (h c) -> p h c", c=64))

            k_ag, v_ag = k_ags[l], v_ags[l]
            if SKIP_COLL:
                nc.sync.dma_start(out=k_ag[0:D, :], in_=k_loc[:, :])
                nc.sync.dma_start(out=v_ag[0:4 * TC, :], in_=v_loc[:, :])
            else:
                nc.gpsimd.collective_compute(
                    "AllGather", ALU.bypass, replica_groups=RG,
                    ins=[k_loc.opt()], outs=[k_ag.opt()])
                nc.gpsimd.collective_compute(
                    "AllGather", ALU.bypass, replica_groups=RG,
                    ins=[v_loc.opt()], outs=[v_ag.opt()])

            # ---- attention, one head-quarter (4 heads) at a time ----
            o_pair = [big_tile(8 + p_, f"opair_{p_}") for p_ in range(8)]
            wq_l = wqT[l].rearrange("(k p) n -> p k n", p=P)
            for hq in range(4):
                # Q projection for heads 4hq..4hq+3, duplicated into both halves
                qh = [big_tile(hh, f"qh_{hh}") for hh in range(4)]
                for oci in range(2):
                    oc = 2 * hq + oci
                    wt = wA.tile([P, 8, P], f32r, name="wq_t", tag="wA")
                    nc.sync.dma_start(out=wt[:], in_=wq_l[:, :, oc * P:(oc + 1) * P])
                    ps = mmtile()
                    for k in range(8):
                        nc.tensor.matmul(out=ps[:], lhsT=wt[:, k, :], rhs=xln[k][:],
                                         start=(k == 0), stop=(k == 7))
                    he, ho = qh[2 * oci], qh[2 * oci + 1]
                    nc.scalar.activation(out=he[0:64, :], in_=ps[0:64, :], func=AF.Copy)
                    nc.scalar.activation(out=ho[64:P, :], in_=ps[64:P, :], func=AF.Copy)
                    nc.sync.dma_start(out=he[64:P, :], in_=he[0:64, :])
                    nc.sync.dma_start(out=ho[0:64, :], in_=ho[64:P, :])
                vts = []
                for kk in range(8):
                    vt = vtp.tile([P, 260], f32r, name=f"vt_{kk}", tag=f"vt_{kk}")
                    nc.gpsimd.indirect_dma_start(
                        out=vt[:], out_offset=None, in_=v_ag[:, :],
                        in_offset=bass.IndirectOffsetOnAxis(
                            ap=vidx_t[:, hq * 8 + kk:hq * 8 + kk + 1], axis=0))
                    vts.append(vt)
                for hh in range(4):
                    h = 4 * hq + hh
                    kt = ktp.tile([P, TC], f32r, name="kt", tag="kt")
                    nc.gpsimd.indirect_dma_start(
                        out=kt[:], out_offset=None, in_=k_ag[:, :],
                        in_offset=bass.IndirectOffsetOnAxis(
                            ap=kidx_t[:, h:h + 1], axis=0))
                    ops = ps_o.tile([65, TC], f32, name="ops", tag="ops")
                    for kk in range(8):
                        lo = kk < 4
                        j = kk % 4
                        qs0 = 0 if lo else QS[j]
                        nq = TC - qs0
                        base = 0 if lo else 64
                        sps = mmtile()
                        nc.tensor.matmul(
                            out=sps[:, 0:nq],
                            lhsT=kt[base:base + 64, j * P:(j + 1) * P],
                            rhs=qh[hh][base:base + 64, qs0:TC],
                            start=True, stop=True)
                        mt = m0_t[j] if lo else m1_t[j]
                        nc.vector.tensor_add(out=sps[:, 0:nq], in0=sps[:, 0:nq],
                                             in1=mt[:, 0:nq])
                        es = esp.tile([P, TC], f32r, name="es", tag="es")
                        nc.scalar.activation(out=es[:, 0:nq], in_=sps[:, 0:nq],
                                             func=AF.Exp, scale=HS ** -0.5)
                        nc.tensor.matmul(
                            out=ops[:, qs0:TC],
                            lhsT=vts[kk][:, 65 * hh:65 * hh + 65],
                            rhs=es[:, 0:nq],
                            start=(kk == 0), stop=(kk == 7))
                    recip = otp.tile([65, TC], f32r, name="recip", tag="recip")
                    with nc.allow_low_precision(reason="f32r softmax denom"):
                        nc.vector.reciprocal(out=recip[64:65, :], in_=ops[64:65, :])
                    bc = ps_bc.tile([P, TC], f32, name="bc_o", tag="bc_a")
                    nc.tensor.matmul(out=bc[0:64, :], lhsT=ones_r[64:65, 0:64],
                                     rhs=recip[64:65, :], start=True, stop=True)
                    bcs = otp.tile([64, TC], f32r, name="bcs", tag="bcs")
                    nc.scalar.activation(out=bcs[:], in_=bc[0:64, :], func=AF.Copy)
                    o_t = otp.tile([64, TC], f32r, name="o_t", tag="o_t")
                    nc.vector.tensor_mul(out=o_t[:], in0=ops[0:64, :], in1=bcs[:])
                    nc.sync.dma_start(
                        out=o_pair[h // 2][64 * (h % 2):64 * (h % 2) + 64, :],
                        in_=o_t[:])

            # ---- attention out projection + residual ----
            wo_l = wo[l].rearrange("(k p) n -> p k n", p=P)
            for dout in range(8):
                wt = wA.tile([P, 8, P], f32r, name="wo_t", tag="wA")
                nc.sync.dma_start(out=wt[:], in_=wo_l[:, :, dout * P:(dout + 1) * P])
                ps = mmtile()
                for k in range(8):
                    nc.tensor.matmul(out=ps[:], lhsT=wt[:, k, :], rhs=o_pair[k][:],
                                     start=(k == 0), stop=(k == 7))
                ysb = sp.tile([P, TC], f32, name="ysb", tag="ysb")
                nc.scalar.activation(out=ysb[:], in_=ps[:], func=AF.Identity,
                                     bias=lnp[:, 32 + dout:33 + dout], scale=1.0)
                nc.vector.tensor_add(out=xr[dout][:], in0=xr[dout][:], in1=ysb[:])

            # ---- FFN ----
            xln2 = xln_tiles()
            layer_norm(lnp, 16, lnp, 24, xln2)

            w1_l = w1[l].rearrange("(k p) n -> p k n", p=P)
            w2_l = w2[l].rearrange("(k p) n -> p k n", p=P)
            y2 = [big_tile(8 + d_, f"y2_{d_}", dtype=f32) for d_ in range(8)]
            for blk in range(4):
                h1 = [big_tile(c_, f"h1_{c_}") for c_ in range(8)]
                for ci in range(8):
                    hc = 8 * blk + ci
                    wt = wA.tile([P, 8, P], f32r, name="w1_t", tag="wA")
                    nc.sync.dma_start(out=wt[:], in_=w1_l[:, :, hc * P:(hc + 1) * P])
                    ps = mmtile()
                    for k in range(8):
                        nc.tensor.matmul(out=ps[:], lhsT=wt[:, k, :], rhs=xln2[k][:],
                                         start=(k == 0), stop=(k == 7))
                    nc.scalar.activation(out=h1[ci][:], in_=ps[:], func=AF.Relu,
                                         bias=b1f_t[:, hc:hc + 1], scale=1.0)
                for dout in range(8):
                    wt = wA.tile([P, 8, P], f32r, name="w2_t", tag="wA")
                    nc.sync.dma_start(
                        out=wt[:],
                        in_=w2_l[:, 8 * blk:8 * blk + 8, dout * P:(dout + 1) * P])
                    ps = mmtile()
                    for c in range(8):
                        nc.tensor.matmul(out=ps[:], lhsT=wt[:, c, :], rhs=h1[c][:],
                                         start=(c == 0), stop=(c == 7))
                    if blk == 0:
                        nc.scalar.activation(out=y2[dout][:], in_=ps[:],
                                             func=AF.Identity,
                                             bias=lnp[:, 40 + dout:41 + dout],
                                             scale=1.0)
                    else:
                        nc.vector.tensor_add(out=y2[dout][:], in0=y2[dout][:],
                                             in1=ps[:])
            for dout in range(8):
                nc.vector.tensor_add(out=xr[dout][:], in0=xr[dout][:],
                                     in1=y2[dout][:])

        # ---- final LN -> xf_loc -> AllGather ----
        lnf_t = sp.tile([P, 16], f32, name="lnf_t", tag="lnp")
        nc.sync.dma_start(out=lnf_t[:, 0:8],
                          in_=lnfg.ap().rearrange("o (j p) -> p (o j)", p=P))
        nc.sync.dma_start(out=lnf_t[:, 8:16],
                          in_=lnfb.ap().rearrange("o (j p) -> p (o j)", p=P))
        xlnf = xln_tiles()
        layer_norm(lnf_t, 0, lnf_t, 8, xlnf)
        for j in range(8):
            nc.sync.dma_start(out=xf_loc[j * P:(j + 1) * P, :], in_=xlnf[j][:])
        if SKIP_COLL:
            nc.sync.dma_start(out=xf_ag[0:D, :], in_=xf_loc[:, :])
        else:
            nc.gpsimd.collective_compute(
                "AllGather", ALU.bypass, replica_groups=RG,
                ins=[xf_loc.opt()], outs=[xf_ag.opt()])

        est.close()

        # ================= LM head (vocab-sharded) =================
        if SKIP_LM:
            outer.close()
            nc.compile()
            return nc
        with tc.tile_pool(name="lmxf", bufs=1) as lmxf, \
             tc.tile_pool(name="lmw", bufs=2) as lmw, \
             tc.tile_pool(name="lmo", bufs=3) as lmo, \
             tc.tile_pool(name="lmb", bufs=2) as lmb, \
             tc.tile_pool(name="ps_lm", bufs=4, space="PSUM") as ps_lm:
            xf_view = xf_ag[:].rearrange("(r j p) w -> p j r w", p=P, j=8)
            xf_t = []
            for j in range(8):
                t = lmxf.tile([P, NCORES, TC], f32r, name=f"xf_{j}", tag=f"xf_{j}")
                nc.sync.dma_start(out=t[:], in_=xf_view[:, j])
                xf_t.append(t[:].rearrange("p r w -> p (r w)"))
            wout_v = wout.rearrange("(k p) n -> p k n", p=P)
            for vs in range(8):
                bt = lmb.tile([1, 500], f32r, name="bt", tag="bt")
                nc.sync.dma_start(out=bt[:], in_=bout[:, vs * 500:(vs + 1) * 500])
                bp = ps_lm.tile([P, TC], f32, name="lm_bps", tag="lm")
                nc.tensor.matmul(out=bp[:, 0:500], lhsT=ones_r[0:1, 0:P],
                                 rhs=bt[:], start=True, stop=True)
                bias_vs = lmb.tile([P, 500], f32, name="bias_vs", tag="bias_vs")
                nc.scalar.activation(out=bias_vs[:], in_=bp[:, 0:500], func=AF.Copy)
                wt = lmw.tile([P, 8, 500], f32r, name="wout_t", tag="wout")
                nc.sync.dma_start(out=wt[:], in_=wout_v[:, :, vs * 500:(vs + 1) * 500])
                for m in range(32):
                    ps = ps_lm.tile([P, TC], f32, name="lm_ps", tag="lm")
                    for k in range(8):
                        nc.tensor.matmul(out=ps[:, 0:500],
                                         lhsT=xf_t[k][:, m * P:(m + 1) * P],
                                         rhs=wt[:, k, :],
                                         start=(k == 0), stop=(k == 7))
                    osb = lmo.tile([P, TC], f32, name="osb", tag="osb")
                    nc.vector.tensor_add(out=osb[:, 0:500], in0=ps[:, 0:500],
                                         in1=bias_vs[:])
                    nc.sync.dma_start(
                        out=out[m * P:(m + 1) * P, vs * 500:(vs + 1) * 500],
                        in_=osb[:, 0:500])
        outer.close()

    nc.compile()
    return nc


def _prep_inputs(inputs):
    """Shard/reformat host inputs into 8 per-core input maps."""
    import ml_dtypes
    inp = {k: np.asarray(v) for k, v in inputs.items()}
    tok = inp['input_tokens'].astype(np.int32)          # [B, T]
    shared = {
        'tokemb': np.ascontiguousarray(inp['tok_emb'], dtype=np.float32),
        'wqT': np.ascontiguousarray(
            inp['Wq'].transpose(0, 2, 1, 3).reshape(L, D, H * HS), dtype=np.float32),
        'wkT': np.ascontiguousarray(
            inp['Wk'].transpose(0, 2, 1, 3).reshape(L, D, H * HS), dtype=np.float32),
        'wvT': np.ascontiguousarray(
            inp['Wv'].transpose(0, 2, 1, 3).reshape(L, D, H * HS), dtype=np.float32),
        'wo': np.ascontiguousarray(inp['Wo'], dtype=np.float32),
        'w1': np.ascontiguousarray(inp['W1'], dtype=np.float32),
        'w2': np.ascontiguousarray(inp['W2'], dtype=np.float32),
        'ln1g': np.ascontiguousarray(inp['ln1_g'], dtype=np.float32),
        'ln1b': np.ascontiguousarray(inp['ln1_b'], dtype=np.float32),
        'ln2g': np.ascontiguousarray(inp['ln2_g'], dtype=np.float32),
        'ln2b': np.ascontiguousarray(inp['ln2_b'], dtype=np.float32),
        'lnfg': np.ascontiguousarray(inp['lnf_g'].reshape(1, D), dtype=np.float32),
        'lnfb': np.ascontiguousarray(inp['lnf_b'].reshape(1, D), dtype=np.float32),
        'bo': np.ascontiguousarray(inp['bo'], dtype=np.float32),
        'b1': np.ascontiguousarray(inp['b1'], dtype=np.float32),
        'b2': np.ascontiguousarray(inp['b2'], dtype=np.float32),
    }
    pe = np.asarray(inp['pos_emb'], dtype=np.float32)
    wout_full = np.asarray(inp['W_out'], dtype=np.float32)
    bout_full = np.asarray(inp['b_out'], dtype=np.float32)

    pcol = np.arange(P)
    in_maps = []
    for c in range(NCORES):
        b, hf = c // 2, c % 2
        m = dict(shared)
        m['tokidx'] = np.ascontiguousarray(
            tok[b, hf * TC:(hf + 1) * TC].reshape(TC, 1))
        m['pos'] = np.ascontiguousarray(pe[hf * TC:(hf + 1) * TC])
        m['wout'] = np.ascontiguousarray(wout_full[:, c * VS:(c + 1) * VS])
        m['bout'] = np.ascontiguousarray(bout_full[c * VS:(c + 1) * VS].reshape(1, VS))
        # K gather indices into k_ag [8*1024, 512]
        kix = np.empty((P, H), np.int32)
        for h in range(H):
            r = 2 * b + (pcol >= 64)
            kix[:, h] = D * r + 64 * h + (pcol % 64)
        m['kidx'] = kix
        # V gather indices into v_ag [8*4*512, 260]
        vix = np.empty((P, 32), np.int32)
        for hq in range(4):
            for kk in range(8):
                r = 2 * b + (1 if kk >= 4 else 0)
                vix[:, hq * 8 + kk] = (4 * TC) * r + TC * hq + P * (kk % 4) + pcol
        m['vidx'] = vix
        # additive masks (bf16-exact values)
        m0 = np.zeros((4, P, TC), np.float32)
        m1 = np.zeros((4, P, TC), np.float32)
        if hf == 0:
            for j in range(4):
                key = P * j + pcol[:, None]
                q = np.arange(TC)[None, :]
                m0[j] = np.where(q >= key, 0.0, NEG)
            m1[:] = NEG
        else:
            for j in range(4):
                qs0 = QS[j]
                key_local = P * j + pcol[:, None]
                q_local = qs0 + np.arange(TC)[None, :]
                m1[j] = np.where(q_local >= key_local, 0.0, NEG)
        m['mask0'] = m0.astype(ml_dtypes.bfloat16)
        m['mask1'] = m1.astype(ml_dtypes.bfloat16)
        in_maps.append(m)
    return in_maps


def _enable_jax_cache():
    try:
        import jax
        jax.config.update("jax_compilation_cache_dir", "/tmp/jax_neff_cache")
        jax.config.update("jax_persistent_cache_min_compile_time_secs", 0.0)
        jax.config.update("jax_persistent_cache_min_entry_size_bytes", -1)
    except Exception:
        pass


def kernel(**inputs):
    global LAST_RESULTS
    _enable_jax_cache()
    from concourse.bass_utils import run_bass_kernel_spmd
    if 'nc' not in _CACHE:
        _CACHE['nc'] = _build()
    nc = _CACHE['nc']
    in_maps = _prep_inputs(inputs)
    kw = {}
    if TRACE:
        kw = dict(trace=True, trace_cores=list(range(NCORES)), stitch_traces=False)
    res = run_bass_kernel_spmd(nc, in_maps, core_ids=list(range(NCORES)), **kw)
    LAST_RESULTS = res
    full = np.concatenate([res.results[c]['out'] for c in range(NCORES)], axis=1)
    return np.ascontiguousarray(full.reshape(B, T, V), dtype=np.float32)



# revision 4
# speedup vs baseline: 1.3778x; 1.3778x over previous
"""Self-contained 8-core Trainium2 kernel for the 6-layer dense transformer.

Sharding: token-parallel. Core c owns batch b=c//2, sequence half h=c%2
(512 tokens). Per layer, each core computes K/V (bf16) for its own tokens and
AllGathers them; causal attention runs locally over the batch's key prefix,
selected from the AG buffer with host-computed indirect-DMA indices. The LM
head is local: every core holds the full (bf16) W_out and computes all 32000
logits for its own 512 tokens — no final collective.

Activations live feature-major (x^T: [D, tokens]). Dense projections run in
float32r; attention internals (Q/K/V tiles, softmax weights, O) run in bf16,
which streams the 64-contraction score and 65-row AV matmuls at full PE rate.
Weights are pre-tiled on the host into the exact SBUF tile layouts so every
weight DMA is partition-contiguous (2KB descriptors). The embedding lookup
(tok_emb gather + pos add + transpose) is done on the host; the device loads
x0^T directly.
"""

import numpy as np

B, T, D, H, HS, L, DFF, V = 4, 1024, 1024, 16, 64, 6, 4096, 32000
NCORES = 8
TC = 512            # tokens per core
P = 128
VB = V // P         # 250 vocab row-blocks
EPS = 1e-5
NEG = -30000.0
QS = [0, 128, 256, 256]   # q-slice starts for key-chunks 4..7 (capped at 256)

_CACHE = {}
TRACE = False
LAST_RESULTS = None


def _build():
    import concourse.bacc as bacc
    import concourse.tile as tile
    import concourse.mybir as mybir
    import concourse.bass as bass
    from contextlib import ExitStack

    f32 = mybir.dt.float32
    f32r = mybir.dt.float32r
    bf16 = mybir.dt.bfloat16
    i32 = mybir.dt.int32
    AF = mybir.ActivationFunctionType
    ALU = mybir.AluOpType

    nc = bacc.Bacc(None, target_bir_lowering=False, debug=False,
                   num_devices=NCORES)

    # ---- parameters (host pre-tiled layouts) ----
    x0T = nc.declare_dram_parameter("x0T", [D, TC], f32, isOutput=False)
    wqp = nc.declare_dram_parameter("wqp", [L, 8, P, 8, P], f32r, isOutput=False)
    wkp = nc.declare_dram_parameter("wkp", [L, 8, P, 8, P], f32r, isOutput=False)
    wvp = nc.declare_dram_parameter("wvp", [L, 4, P, 8, 256], f32r, isOutput=False)
    wop = nc.declare_dram_parameter("wop", [L, 8, P, 8, P], bf16, isOutput=False)
    w1p = nc.declare_dram_parameter("w1p", [L, 32, P, 8, P], f32r, isOutput=False)
    w2p = nc.declare_dram_parameter("w2p", [L, 4, 8, P, 8, P], f32r, isOutput=False)
    woutp = nc.declare_dram_parameter("woutp", [VB, P, 8, P], bf16, isOutput=False)
    boutp = nc.declare_dram_parameter("boutp", [P, VB], f32, isOutput=False)
    lnpp = nc.declare_dram_parameter("lnpp", [L, P, 48], f32, isOutput=False)
    b1p = nc.declare_dram_parameter("b1p", [L, P, 32], f32, isOutput=False)
    lnfp = nc.declare_dram_parameter("lnfp", [P, 16], f32, isOutput=False)
    mask0 = nc.declare_dram_parameter("mask0", [4, P, TC], bf16, isOutput=False)
    mask1 = nc.declare_dram_parameter("mask1", [4, P, TC], bf16, isOutput=False)
    kidx = nc.declare_dram_parameter("kidx", [P, H], i32, isOutput=False)
    vidx = nc.declare_dram_parameter("vidx", [P, 32], i32, isOutput=False)
    out = nc.declare_dram_parameter("out", [V, TC], f32, isOutput=True)

    RG = [list(range(NCORES))]

    with tile.TileContext(nc) as tc:
        outer = ExitStack()
        singles = outer.enter_context(tc.tile_pool(name="singles", bufs=1))
        dramp = outer.enter_context(tc.tile_pool(name="dramp", bufs=1, space="DRAM"))

        # ---- internal DRAM ----
        k_loc = dramp.tile([D, TC], bf16, name="k_loc")
        v_loc = dramp.tile([4 * TC, 260], bf16, name="v_loc")
        k_ags = [dramp.tile([NCORES * D, TC], bf16, name=f"k_ag_{i}",
                            addr_space="Shared") for i in range(L)]
        v_ags = [dramp.tile([NCORES * 4 * TC, 260], bf16, name=f"v_ag_{i}",
                            addr_space="Shared") for i in range(L)]

        # constants
        ones_f = singles.tile([P, 144], f32, name="ones_f")
        nc.vector.memset(ones_f[:], 1.0)
        ones_r = singles.tile([P, 144], f32r, name="ones_r")
        nc.vector.tensor_copy(out=ones_r[:], in_=ones_f[:])
        ones_bf = singles.tile([P, 8], bf16, name="ones_bf")
        nc.vector.memset(ones_bf[:], 1.0)
        eps_t = singles.tile([1, 1], f32, name="eps_t")
        nc.vector.memset(eps_t[:], EPS)
        kidx_t = singles.tile([P, H], i32, name="kidx_t")
        nc.sync.dma_start(out=kidx_t[:], in_=kidx[:])
        vidx_t = singles.tile([P, 32], i32, name="vidx_t")
        nc.sync.dma_start(out=vidx_t[:], in_=vidx[:])
        bout_t = singles.tile([P, VB], f32, name="bout_t")
        nc.sync.dma_start(out=bout_t[:], in_=boutp[:])
        m0_t = []
        m1_t = []
        for j in range(4):
            mt = singles.tile([P, TC], bf16, name=f"m0_{j}")
            nc.sync.dma_start(out=mt[:], in_=mask0[j])
            m0_t.append(mt)
            mt = singles.tile([P, TC], bf16, name=f"m1_{j}")
            nc.sync.dma_start(out=mt[:], in_=mask1[j])
            m1_t.append(mt)

        est = ExitStack()
        lp = est.enter_context(tc.tile_pool(name="lp", bufs=1))      # xr/xln
        big = est.enter_context(tc.tile_pool(name="big", bufs=1))    # 16 shared slots
        qp = est.enter_context(tc.tile_pool(name="qp", bufs=1))      # 16 bf16 q tiles
        wA = est.enter_context(tc.tile_pool(name="wA", bufs=3))      # [P,8,128] weights
        wB = est.enter_context(tc.tile_pool(name="wB", bufs=2))      # [P,8,256] wv quarters
        sp = est.enter_context(tc.tile_pool(name="sp", bufs=2))      # stream tiles
        kvp = est.enter_context(tc.tile_pool(name="kvp", bufs=2))    # kv copyback
        ktp = est.enter_context(tc.tile_pool(name="ktp", bufs=2))    # K gathers
        esp = est.enter_context(tc.tile_pool(name="esp", bufs=3))    # exp(scores)
        vtp = est.enter_context(tc.tile_pool(name="vtp", bufs=1))    # V gathers (8 tags)
        otp = est.enter_context(tc.tile_pool(name="otp", bufs=2))    # o tmp / recip
        stp = est.enter_context(tc.tile_pool(name="stp", bufs=1))    # LN stats [1,*]

        ps_mm = est.enter_context(tc.tile_pool(name="ps_mm", bufs=2, space="PSUM"))
        ps_o = est.enter_context(tc.tile_pool(name="ps_o", bufs=2, space="PSUM"))
        ps_st = est.enter_context(tc.tile_pool(name="ps_st", bufs=1, space="PSUM"))
        ps_bc = est.enter_context(tc.tile_pool(name="ps_bc", bufs=1, space="PSUM"))

        def mmtile():
            return ps_mm.tile([P, TC], f32, name="mm", tag="mm")

        xr = [lp.tile([P, TC], f32, name=f"xr_{j}", tag=f"xr_{j}") for j in range(8)]

        def xln_tiles(dtype=f32r):
            return [lp.tile([P, TC], dtype, name=f"xln_{j}", tag=f"xln_{j}")
                    for j in range(8)]

        def big_tile(i, name, dtype=f32r):
            return big.tile([P, TC], dtype, name=name, tag=f"big_{i}")

        # ---- embedding: host-precomputed x0T, straight loads ----
        for j in range(8):
            nc.sync.dma_start(out=xr[j][:], in_=x0T[j * P:(j + 1) * P, :])

        # ---- ones columns of v_loc (V writes never touch them) ----
        vl_view = v_loc[:].rearrange("(hq t) (h c) -> hq t h c", hq=4, c=65)
        for hq in range(4):
            for t4 in range(4):
                nc.sync.dma_start(
                    out=vl_view[hq, t4 * P:(t4 + 1) * P, 0:4, 64:65],
                    in_=ones_bf[:, 0:4])

        def layer_norm(g_t, gcol, b_t, bcol, out_tiles):
            """xr (f32) -> out_tiles; feature-major LN over partitions."""
            sum_ps = ps_st.tile([1, TC], f32, name="sum_ps", tag="st_a")
            sumsq_ps = ps_st.tile([1, TC], f32, name="sumsq_ps", tag="st_b")
            for j in range(8):
                xc = sp.tile([P, TC], f32r, name="ln_xc", tag="ln_xc")
                nc.scalar.activation(out=xc[:], in_=xr[j][:], func=AF.Copy)
                sq = sp.tile([P, TC], f32r, name="ln_sq", tag="ln_sq")
                nc.scalar.activation(out=sq[:], in_=xr[j][:], func=AF.Square)
                nc.tensor.matmul(out=sum_ps[:], lhsT=ones_r[:, 0:1], rhs=xc[:],
                                 start=(j == 0), stop=(j == 7))
                nc.tensor.matmul(out=sumsq_ps[:], lhsT=ones_r[:, 1:2], rhs=sq[:],
                                 start=(j == 0), stop=(j == 7))
            nmean = stp.tile([1, TC], f32r, name="ln_nmean", tag="ln_nmean")
            with nc.allow_low_precision(reason="f32r LN stats"):
                nc.scalar.activation(out=nmean[:], in_=sum_ps[:], func=AF.Copy,
                                     scale=-1.0 / D)
            ms = stp.tile([1, TC], f32, name="ln_ms", tag="ln_ms")
            nc.scalar.activation(out=ms[:], in_=sumsq_ps[:], func=AF.Copy,
                                 scale=1.0 / D)
            m2 = stp.tile([1, TC], f32, name="ln_m2", tag="ln_m2")
            nc.vector.tensor_mul(out=m2[:], in0=nmean[:], in1=nmean[:])
            var = stp.tile([1, TC], f32, name="ln_var", tag="ln_var")
            nc.vector.tensor_tensor(out=var[:], in0=ms[:], in1=m2[:],
                                    op=ALU.subtract)
            std = stp.tile([1, TC], f32r, name="ln_std", tag="ln_std")
            with nc.allow_low_precision(reason="f32r LN stats"):
                nc.scalar.activation(out=std[:], in_=var[:], func=AF.Sqrt,
                                     bias=eps_t[:], scale=1.0)
            # broadcast std and -mean along partitions, then full-width recip
            bc_s = ps_bc.tile([P, TC], f32, name="bc_s", tag="bc_a")
            nc.tensor.matmul(out=bc_s[:], lhsT=ones_r[0:1, 0:P], rhs=std[:],
                             start=True, stop=True)
            bc_m = ps_bc.tile([P, TC], f32, name="bc_m", tag="bc_c")
            nc.tensor.matmul(out=bc_m[:], lhsT=ones_r[0:1, 0:P], rhs=nmean[:],
                             start=True, stop=True)
            rstd_bc = sp.tile([P, TC], f32, name="ln_rb", tag="ln_rb")
            nc.vector.reciprocal(out=rstd_bc[:], in_=bc_s[:])
            for j in range(8):
                t1 = sp.tile([P, TC], f32, name="ln_t1", tag="ln_t1")
                nc.vector.tensor_add(out=t1[:], in0=xr[j][:], in1=bc_m[:])
                nc.vector.tensor_mul(out=t1[:], in0=t1[:], in1=rstd_bc[:])
                nc.vector.tensor_scalar(
                    out=out_tiles[j][:], in0=t1[:],
                    scalar1=g_t[:, gcol + j:gcol + j + 1],
                    scalar2=b_t[:, bcol + j:bcol + j + 1],
                    op0=ALU.mult, op1=ALU.add)

        # ================= layers =================
        for l in range(L):
            lnp = sp.tile([P, 48], f32, name="lnp", tag="lnp")
            nc.sync.dma_start(out=lnp[:], in_=lnpp[l])
            b1f_t = sp.tile([P, 32], f32, name="b1f_t", tag="b1f_t")
            nc.sync.dma_start(out=b1f_t[:], in_=b1p[l])

            xln = xln_tiles()
            layer_norm(lnp, 0, lnp, 8, xln)

            # ---- K projection -> k_loc (bf16) ----
            for oc in range(8):
                wt = wA.tile([P, 8, P], f32r, name="wk_t", tag="wA")
                nc.sync.dma_start(out=wt[:], in_=wkp[l, oc])
                ps = mmtile()
                for k in range(8):
                    nc.tensor.matmul(out=ps[:], lhsT=wt[:, k, :], rhs=xln[k][:],
                                     start=(k == 0), stop=(k == 7))
                ksb = kvp.tile([P, TC], bf16, name="ksb", tag="ksb")
                nc.scalar.activation(out=ksb[:], in_=ps[:], func=AF.Copy)
                nc.sync.dma_start(out=k_loc[oc * P:(oc + 1) * P, :], in_=ksb[:])

            k_ag, v_ag = k_ags[l], v_ags[l]
            nc.gpsimd.collective_compute(
                "AllGather", ALU.bypass, replica_groups=RG,
                ins=[k_loc.opt()], outs=[k_ag.opt()])

            # ---- V projection -> v_loc (bf16, head-quarter rows, 65-strided) ----
            for hq in range(4):
                wt = wB.tile([P, 8, 256], f32r, name="wv_t", tag="wB")
                nc.sync.dma_start(out=wt[:], in_=wvp[l, hq])
                for t4 in range(4):
                    ps = mmtile()
                    for k in range(8):
                        nc.tensor.matmul(out=ps[:, 0:256],
                                         lhsT=xln[k][:, t4 * P:(t4 + 1) * P],
                                         rhs=wt[:, k, :],
                                         start=(k == 0), stop=(k == 7))
                    vsb = kvp.tile([P, 256], bf16, name="vsb", tag="vsb")
                    nc.scalar.activation(out=vsb[:], in_=ps[:, 0:256], func=AF.Copy)
                    nc.sync.dma_start(
                        out=vl_view[hq, t4 * P:(t4 + 1) * P, 0:4, 0:64],
                        in_=vsb[:].rearrange("p (h c) -> p h c", c=64))

            nc.gpsimd.collective_compute(
                "AllGather", ALU.bypass, replica_groups=RG,
                ins=[v_loc.opt()], outs=[v_ag.opt()])

            # ---- Q projection for all 16 heads (overlaps the V AllGather) ----
            qh = [[qp.tile([P, TC], bf16, name=f"qh_{hq}_{hh}", tag=f"qh_{hq}_{hh}")
                   for hh in range(4)] for hq in range(4)]
            for hq in range(4):
                for oci in range(2):
                    oc = 2 * hq + oci
                    wt = wA.tile([P, 8, P], f32r, name="wq_t", tag="wA")
                    nc.sync.dma_start(out=wt[:], in_=wqp[l, oc])
                    ps = mmtile()
                    for k in range(8):
                        nc.tensor.matmul(out=ps[:], lhsT=wt[:, k, :], rhs=xln[k][:],
                                         start=(k == 0), stop=(k == 7))
                    he, ho = qh[hq][2 * oci], qh[hq][2 * oci + 1]
                    nc.scalar.activation(out=he[0:64, :], in_=ps[0:64, :], func=AF.Copy)
                    nc.scalar.activation(out=ho[64:P, :], in_=ps[64:P, :], func=AF.Copy)
                    nc.sync.dma_start(out=he[64:P, :], in_=he[0:64, :])
                    nc.sync.dma_start(out=ho[0:64, :], in_=ho[64:P, :])

            # ---- attention, one head-quarter (4 heads) at a time ----
            o_pair = [big_tile(8 + p_, f"opair_{p_}", dtype=bf16) for p_ in range(8)]
            for hq in range(4):
                vts = []
                for kk in range(8):
                    vt = vtp.tile([P, 260], bf16, name=f"vt_{kk}", tag=f"vt_{kk}")
                    nc.gpsimd.indirect_dma_start(
                        out=vt[:], out_offset=None, in_=v_ag[:, :],
                        in_offset=bass.IndirectOffsetOnAxis(
                            ap=vidx_t[:, hq * 8 + kk:hq * 8 + kk + 1], axis=0))
                    vts.append(vt)
                for hh in range(4):
                    h = 4 * hq + hh
                    kt = ktp.tile([P, TC], bf16, name="kt", tag="kt")
                    nc.gpsimd.indirect_dma_start(
                        out=kt[:], out_offset=None, in_=k_ag[:, :],
                        in_offset=bass.IndirectOffsetOnAxis(
                            ap=kidx_t[:, h:h + 1], axis=0))
                    ops = ps_o.tile([65, TC], f32, name="ops", tag="ops")
                    for kk in range(8):
                        lo = kk < 4
                        j = kk % 4
                        qs0 = 0 if lo else QS[j]
                        nq = TC - qs0
                        base = 0 if lo else 64
                        sps = mmtile()
                        nc.tensor.matmul(
                            out=sps[:, 0:nq],
                            lhsT=kt[base:base + 64, j * P:(j + 1) * P],
                            rhs=qh[hq][hh][base:base + 64, qs0:TC],
                            start=True, stop=True)
                        mt = m0_t[j] if lo else m1_t[j]
                        nc.vector.tensor_add(out=sps[:, 0:nq], in0=sps[:, 0:nq],
                                             in1=mt[:, 0:nq])
                        es = esp.tile([P, TC], bf16, name="es", tag="es")
                        nc.scalar.activation(out=es[:, 0:nq], in_=sps[:, 0:nq],
                                             func=AF.Exp, scale=HS ** -0.5)
                        nc.tensor.matmul(
                            out=ops[:, qs0:TC],
                            lhsT=vts[kk][:, 65 * hh:65 * hh + 65],
                            rhs=es[:, 0:nq],
                            start=(kk == 0), stop=(kk == 7))
                    dsb = otp.tile([1, TC], f32r, name="dsb", tag="recip")
                    with nc.allow_low_precision(reason="f32r softmax denom"):
                        nc.scalar.activation(out=dsb[:], in_=ops[64:65, :],
                                             func=AF.Copy)
                    bc = ps_bc.tile([P, TC], f32, name="bc_o", tag="bc_a")
                    nc.tensor.matmul(out=bc[0:64, :], lhsT=ones_r[0:1, 0:64],
                                     rhs=dsb[:], start=True, stop=True)
                    bcr = otp.tile([64, TC], bf16, name="bcr", tag="bcs")
                    with nc.allow_low_precision(reason="bf16 softmax denom recip"):
                        nc.vector.reciprocal(out=bcr[:], in_=bc[0:64, :])
                    o_t = otp.tile([64, TC], bf16, name="o_t", tag="o_t")
                    nc.vector.tensor_mul(out=o_t[:], in0=ops[0:64, :], in1=bcr[:])
                    nc.sync.dma_start(
                        out=o_pair[h // 2][64 * (h % 2):64 * (h % 2) + 64, :],
                        in_=o_t[:])

            # ---- attention out projection + residual ----
            for dout in range(8):
                wt = wA.tile([P, 8, P], bf16, name="wo_t", tag="wA")
                nc.sync.dma_start(out=wt[:], in_=wop[l, dout])
                ps = mmtile()
                for k in range(8):
                    nc.tensor.matmul(out=ps[:], lhsT=wt[:, k, :], rhs=o_pair[k][:],
                                     start=(k == 0), stop=(k == 7))
                ysb = sp.tile([P, TC], f32, name="ysb", tag="ysb")
                nc.scalar.activation(out=ysb[:], in_=ps[:], func=AF.Identity,
                                     bias=lnp[:, 32 + dout:33 + dout], scale=1.0)
                nc.vector.tensor_add(out=xr[dout][:], in0=xr[dout][:], in1=ysb[:])

            # ---- FFN ----
            xln2 = xln_tiles()
            layer_norm(lnp, 16, lnp, 24, xln2)

            y2 = [big_tile(8 + d_, f"y2_{d_}", dtype=f32) for d_ in range(8)]
            for blk in range(4):
                h1 = [big_tile(c_, f"h1_{c_}") for c_ in range(8)]
                for ci in range(8):
                    hc = 8 * blk + ci
                    wt = wA.tile([P, 8, P], f32r, name="w1_t", tag="wA")
                    nc.sync.dma_start(out=wt[:], in_=w1p[l, hc])
                    ps = mmtile()
                    for k in range(8):
                        nc.tensor.matmul(out=ps[:], lhsT=wt[:, k, :], rhs=xln2[k][:],
                                         start=(k == 0), stop=(k == 7))
                    nc.scalar.activation(out=h1[ci][:], in_=ps[:], func=AF.Relu,
                                         bias=b1f_t[:, hc:hc + 1], scale=1.0)
                for dout in range(8):
                    wt = wA.tile([P, 8, P], f32r, name="w2_t", tag="wA")
                    nc.sync.dma_start(out=wt[:], in_=w2p[l, blk, dout])
                    ps = mmtile()
                    for c in range(8):
                        nc.tensor.matmul(out=ps[:], lhsT=wt[:, c, :], rhs=h1[c][:],
                                         start=(c == 0), stop=(c == 7))
                    if blk == 0:
                        nc.scalar.activation(out=y2[dout][:], in_=ps[:],
                                             func=AF.Identity,
                                             bias=lnp[:, 40 + dout:41 + dout],
                                             scale=1.0)
                    else:
                        nc.vector.tensor_add(out=y2[dout][:], in0=y2[dout][:],
                                             in1=ps[:])
            for dout in range(8):
                nc.vector.tensor_add(out=xr[dout][:], in0=xr[dout][:],
                                     in1=y2[dout][:])

        # ---- final LN (bf16 output for the LM head) ----
        lnf_t = sp.tile([P, 16], f32, name="lnf_t", tag="lnp")
        nc.sync.dma_start(out=lnf_t[:], in_=lnfp[:])
        xlnf = xln_tiles(dtype=bf16)
        layer_norm(lnf_t, 0, lnf_t, 8, xlnf)

        est.close()

        # ================= LM head (local, full vocab) =================
        with tc.tile_pool(name="lmw", bufs=3) as lmw, \
             tc.tile_pool(name="lmo", bufs=3) as lmo, \
             tc.tile_pool(name="ps_lm", bufs=4, space="PSUM") as ps_lm:
            for vb in range(VB):
                wt = lmw.tile([P, 8, P], bf16, name="wout_t", tag="wout")
                nc.sync.dma_start(out=wt[:], in_=woutp[vb])
                ps = ps_lm.tile([P, TC], f32, name="lm_ps", tag="lm")
                for k in range(8):
                    nc.tensor.matmul(out=ps[:], lhsT=wt[:, k, :], rhs=xlnf[k][:],
                                     start=(k == 0), stop=(k == 7))
                osb = lmo.tile([P, TC], f32, name="osb", tag="osb")
                nc.scalar.activation(out=osb[:], in_=ps[:], func=AF.Identity,
                                     bias=bout_t[:, vb:vb + 1], scale=1.0)
                nc.sync.dma_start(out=out[vb * P:(vb + 1) * P, :], in_=osb[:])
        outer.close()

    nc.compile()
    return nc


def _prep_inputs(inputs):
    """Shard/reformat host inputs into 8 per-core input maps."""
    import ml_dtypes
    bf = ml_dtypes.bfloat16
    inp = {k: np.asarray(v) for k, v in inputs.items()}
    tok = inp['input_tokens'].astype(np.int64)          # [B, T]
    temb = np.asarray(inp['tok_emb'], dtype=np.float32)
    pe = np.asarray(inp['pos_emb'], dtype=np.float32)

    def tile_st(w, no, nk):
        # [L, D, no*128-ish] -> [L, no, P, nk, n] stationary tile layout
        Lc, Dc, M = w.shape
        n = M // no
        return np.ascontiguousarray(
            w.reshape(Lc, nk, P, no, n).transpose(0, 3, 2, 1, 4))

    WqT = inp['Wq'].transpose(0, 2, 1, 3).reshape(L, D, H * HS)
    WkT = inp['Wk'].transpose(0, 2, 1, 3).reshape(L, D, H * HS)
    WvT = inp['Wv'].transpose(0, 2, 1, 3).reshape(L, D, H * HS)
    shared = {
        'wqp': tile_st(np.asarray(WqT, np.float32), 8, 8),
        'wkp': tile_st(np.asarray(WkT, np.float32), 8, 8),
        'wvp': tile_st(np.asarray(WvT, np.float32), 4, 8),
        'wop': tile_st(np.asarray(inp['Wo'], np.float32), 8, 8).astype(bf),
        'w1p': tile_st(np.asarray(inp['W1'], np.float32), 32, 8),
        'w2p': np.ascontiguousarray(
            np.asarray(inp['W2'], np.float32)
            .reshape(L, 4, 8, P, 8, P).transpose(0, 1, 4, 3, 2, 5)),
        'woutp': np.ascontiguousarray(
            np.asarray(inp['W_out'], np.float32)
            .reshape(8, P, VB, P).transpose(2, 1, 0, 3)).astype(bf),
        'boutp': np.ascontiguousarray(
            np.asarray(inp['b_out'], np.float32).reshape(VB, P).T),
        'lnpp': np.ascontiguousarray(
            np.stack([inp['ln1_g'], inp['ln1_b'], inp['ln2_g'], inp['ln2_b'],
                      inp['bo'], inp['b2']], axis=1)      # [L, 6, D]
            .reshape(L, 6, 8, P).transpose(0, 3, 1, 2)    # [L, P, 6, 8]
            .reshape(L, P, 48).astype(np.float32)),
        'b1p': np.ascontiguousarray(
            np.asarray(inp['b1'], np.float32).reshape(L, 32, P).transpose(0, 2, 1)),
        'lnfp': np.ascontiguousarray(
            np.stack([inp['lnf_g'], inp['lnf_b']], axis=0)  # [2, D]
            .reshape(2, 8, P).transpose(2, 0, 1).reshape(P, 16).astype(np.float32)),
    }
    # lnpp col order check: cols j*... we want cols 0:8=ln1g etc.
    # layout above gives [P, param, j] -> col index = param*8 + j  (matches
    # gcol=0,8,16,24 and bias cols 32..48 used in the kernel)

    pcol = np.arange(P)
    in_maps = []
    for c in range(NCORES):
        b, hf = c // 2, c % 2
        m = dict(shared)
        toks = tok[b, hf * TC:(hf + 1) * TC]
        x0 = temb[toks] + pe[hf * TC:(hf + 1) * TC]      # [TC, D]
        m['x0T'] = np.ascontiguousarray(x0.T, dtype=np.float32)
        # K gather indices into k_ag [8*1024, 512]
        kix = np.empty((P, H), np.int32)
        for h in range(H):
            r = 2 * b + (pcol >= 64)
            kix[:, h] = D * r + 64 * h + (pcol % 64)
        m['kidx'] = kix
        # V gather indices into v_ag [8*4*512, 260]
        vix = np.empty((P, 32), np.int32)
        for hq in range(4):
            for kk in range(8):
                r = 2 * b + (1 if kk >= 4 else 0)
                vix[:, hq * 8 + kk] = (4 * TC) * r + TC * hq + P * (kk % 4) + pcol
        m['vidx'] = vix
        # additive masks (bf16-exact values)
        m0 = np.zeros((4, P, TC), np.float32)
        m1 = np.zeros((4, P, TC), np.float32)
        if hf == 0:
            for j in range(4):
                key = P * j + pcol[:, None]
                q = np.arange(TC)[None, :]
                m0[j] = np.where(q >= key, 0.0, NEG)
            m1[:] = NEG
        else:
            for j in range(4):
                qs0 = QS[j]
                key_local = P * j + pcol[:, None]
                q_local = qs0 + np.arange(TC)[None, :]
                m1[j] = np.where(q_local >= key_local, 0.0, NEG)
        m['mask0'] = m0.astype(bf)
        m['mask1'] = m1.astype(bf)
        in_maps.append(m)
    return in_maps


def _enable_jax_cache():
    try:
        import jax
        jax.config.update("jax_compilation_cache_dir", "/tmp/jax_neff_cache")
        jax.config.update("jax_persistent_cache_min_compile_time_secs", 0.0)
        jax.config.update("jax_persistent_cache_min_entry_size_bytes", -1)
    except Exception:
        pass


def kernel(**inputs):
    global LAST_RESULTS
    _enable_jax_cache()
    from concourse.bass_utils import run_bass_kernel_spmd
    if 'nc' not in _CACHE:
        _CACHE['nc'] = _build()
    nc = _CACHE['nc']
    in_maps = _prep_inputs(inputs)
    kw = {}
    if TRACE:
        kw = dict(trace=True, trace_cores=list(range(NCORES)), stitch_traces=False)
    res = run_bass_kernel_spmd(nc, in_maps, core_ids=list(range(NCORES)), **kw)
    LAST_RESULTS = res
    full = np.empty((B, T, V), np.float32)
    for c in range(NCORES):
        b, hf = c // 2, c % 2
        full[b, hf * TC:(hf + 1) * TC, :] = res.results[c]['out'].T
    return full


# revision 8
# speedup vs baseline: 1.8191x; 1.3202x over previous
"""Self-contained 8-core Trainium2 kernel for the 6-layer dense transformer.

Sharding: batch pairs with head-split attention. Core c owns batch b=c//2 and
sequence half hf=c%2 (512 tokens) for the residual stream, LayerNorms, FFN and
LM head. Attention for batch b is split by heads across the pair: core 2b
computes heads 0-7, core 2b+1 heads 8-15, each over all 1024 tokens, so K/V
never cross cores. The only collectives are pairwise: an AllGather of the LN1
output (so both cores see all 1024 tokens) and a ReduceScatter of the Wo
partial products (each core receives the summed attention output for its own
tokens). The LM head is local: every core holds the full bf16 W_out.

Activations live feature-major (x^T: [D, tokens]); residual stays f32,
LN statistics f32r, everything else (projections, attention, FFN, LM head)
bf16 with f32 PSUM accumulation. Weights are pre-tiled on the host so every
weight DMA is partition-contiguous. Softmax denominators are inverted via
exp(-ln(d)) on the scalar engine (DVE reciprocal is ~25x slower per element).
The embedding lookup runs on the host; the device loads x0^T directly.
"""

import numpy as np

B, T, D, H, HS, L, DFF, V = 4, 1024, 1024, 16, 64, 6, 4096, 32000
NCORES = 8
TC = 512            # tokens owned per core
TB = 1024           # tokens per batch (attention span)
P = 128
VB = V // P         # 250 vocab row-blocks
EPS = 1e-5

_CACHE = {}
TRACE = False
LAST_RESULTS = None


def _build():
    import concourse.bacc as bacc
    import concourse.tile as tile
    import concourse.mybir as mybir
    from contextlib import ExitStack

    f32 = mybir.dt.float32
    f32r = mybir.dt.float32r
    bf16 = mybir.dt.bfloat16
    AF = mybir.ActivationFunctionType
    ALU = mybir.AluOpType

    nc = bacc.Bacc(None, target_bir_lowering=False, debug=False,
                   num_devices=NCORES)

    # ---- parameters (host pre-tiled layouts; wq/wk/wv/wo/lnpp per-core) ----
    x0T = nc.declare_dram_parameter("x0T", [D, TC], f32, isOutput=False)
    wqp = nc.declare_dram_parameter("wqp", [L, P, 8, 4, P], bf16, isOutput=False)
    wkp = nc.declare_dram_parameter("wkp", [L, P, 8, 4, P], bf16, isOutput=False)
    wvp = nc.declare_dram_parameter("wvp", [L, P, 8, 512], bf16, isOutput=False)
    wop = nc.declare_dram_parameter("wop", [L, P, 4, 8, P], bf16, isOutput=False)
    w1p = nc.declare_dram_parameter("w1p", [L, 32, P, 8, P], bf16, isOutput=False)
    w2p = nc.declare_dram_parameter("w2p", [L, 4, 8, P, 8, P], bf16, isOutput=False)
    woutp = nc.declare_dram_parameter("woutp", [VB, P, 8, P], bf16, isOutput=False)
    boutp = nc.declare_dram_parameter("boutp", [P, VB], f32, isOutput=False)
    lnpp = nc.declare_dram_parameter("lnpp", [L, P, 48], f32, isOutput=False)
    b1p = nc.declare_dram_parameter("b1p", [L, P, 32], f32, isOutput=False)
    lnfp = nc.declare_dram_parameter("lnfp", [P, 16], f32, isOutput=False)
    cmask = nc.declare_dram_parameter("cmask", [4, P, TC], bf16, isOutput=False)
    out = nc.declare_dram_parameter("out", [V, TC], f32, isOutput=True)

    RG2 = [[0, 1], [2, 3], [4, 5], [6, 7]]

    with tile.TileContext(nc) as tc:
        outer = ExitStack()
        singles = outer.enter_context(tc.tile_pool(name="singles", bufs=1))
        dramp = outer.enter_context(tc.tile_pool(name="dramp", bufs=1, space="DRAM"))

        # ---- internal DRAM (per layer, to keep WAR edges off the schedule) ----
        xln_locs = [dramp.tile([D, TC], bf16, name=f"xln_loc_{i}") for i in range(L)]
        xln_pairs = [dramp.tile([2 * D, TC], bf16, name=f"xln_pair_{i}")
                     for i in range(L)]
        part_locs = [dramp.tile([2 * D, TC], bf16, name=f"part_loc_{i}")
                     for i in range(L)]
        attn_rss = [dramp.tile([D, TC], bf16, name=f"attn_rs_{i}") for i in range(L)]

        # constants
        ones_f = singles.tile([P, 144], f32, name="ones_f")
        nc.vector.memset(ones_f[:], 1.0)
        ones_r = singles.tile([P, 144], f32r, name="ones_r")
        nc.vector.tensor_copy(out=ones_r[:], in_=ones_f[:])
        eps_t = singles.tile([1, 1], f32, name="eps_t")
        nc.vector.memset(eps_t[:], EPS)
        bout_t = singles.tile([P, VB], f32, name="bout_t")
        nc.sync.dma_start(out=bout_t[:], in_=boutp[:])
        cm_t = []
        for j in range(4):
            mt = singles.tile([P, TC], bf16, name=f"cm_{j}")
            nc.sync.dma_start(out=mt[:], in_=cmask[j])
            cm_t.append(mt)

        est = ExitStack()
        lp = est.enter_context(tc.tile_pool(name="lp", bufs=1))      # xr/xln tiles
        kqp = est.enter_context(tc.tile_pool(name="kqp", bufs=1))    # K/Q [P,1024]
        vtsp = est.enter_context(tc.tile_pool(name="vtsp", bufs=1))  # V 65-strided
        osp = est.enter_context(tc.tile_pool(name="osp", bufs=1))    # o [P,1024]
        big = est.enter_context(tc.tile_pool(name="big", bufs=1))    # h1/y2 slots
        wL = est.enter_context(tc.tile_pool(name="wL", bufs=1))      # layer weights
        wA = est.enter_context(tc.tile_pool(name="wA", bufs=3))      # ffn stream
        sp = est.enter_context(tc.tile_pool(name="sp", bufs=2))      # stream tiles
        esp = est.enter_context(tc.tile_pool(name="esp", bufs=2))    # exp(scores)
        otp = est.enter_context(tc.tile_pool(name="otp", bufs=2))    # denom tmp
        stp = est.enter_context(tc.tile_pool(name="stp", bufs=1))    # LN stats [1,*]

        ps_mm = est.enter_context(tc.tile_pool(name="ps_mm", bufs=2, space="PSUM"))
        ps_o = est.enter_context(tc.tile_pool(name="ps_o", bufs=1, space="PSUM"))
        ps_st = est.enter_context(tc.tile_pool(name="ps_st", bufs=1, space="PSUM"))
        ps_bc = est.enter_context(tc.tile_pool(name="ps_bc", bufs=1, space="PSUM"))

        def mmtile():
            return ps_mm.tile([P, TC], f32, name="mm", tag="mm")

        xr = [lp.tile([P, TC], f32, name=f"xr_{j}", tag=f"xr_{j}") for j in range(8)]

        def own_tiles(dtype=bf16):
            return [lp.tile([P, TC], dtype, name=f"xln_{j}", tag=f"xln_{j}")
                    for j in range(8)]

        # persistent attention tiles
        kq_k = [kqp.tile([P, TB], bf16, name=f"kk_{i}", tag=f"kk_{i}")
                for i in range(4)]
        kq_q = [kqp.tile([P, TB], bf16, name=f"kq_{i}", tag=f"kq_{i}")
                for i in range(4)]
        vts = [vtsp.tile([P, 520], bf16, name=f"vts_{i}", tag=f"vts_{i}")
               for i in range(8)]
        for i in range(8):
            nc.vector.memset(vts[i][:], 1.0)   # ones cols (64 of each 65) persist
        o_sb = [osp.tile([P, TB], bf16, name=f"osb_{i}", tag=f"osb_{i}")
                for i in range(4)]

        def big_tile(i, name, dtype=bf16):
            return big.tile([P, TC], dtype, name=name, tag=f"big_{i}")

        # ---- embedding: host-precomputed x0T, straight loads ----
        for j in range(8):
            nc.sync.dma_start(out=xr[j][:], in_=x0T[j * P:(j + 1) * P, :])

        def layer_norm(g_t, gcol, b_t, bcol, out_tiles):
            """xr (f32) -> out_tiles; feature-major LN over partitions."""
            sum_ps = ps_st.tile([1, TC], f32, name="sum_ps", tag="st_a")
            sumsq_ps = ps_st.tile([1, TC], f32, name="sumsq_ps", tag="st_b")
            for j in range(8):
                xc = sp.tile([P, TC], f32r, name="ln_xc", tag="ln_xc")
                nc.scalar.activation(out=xc[:], in_=xr[j][:], func=AF.Copy)
                sq = sp.tile([P, TC], f32r, name="ln_sq", tag="ln_sq")
                nc.scalar.activation(out=sq[:], in_=xr[j][:], func=AF.Square)
                nc.tensor.matmul(out=sum_ps[:], lhsT=ones_r[:, 0:1], rhs=xc[:],
                                 start=(j == 0), stop=(j == 7))
                nc.tensor.matmul(out=sumsq_ps[:], lhsT=ones_r[:, 1:2], rhs=sq[:],
                                 start=(j == 0), stop=(j == 7))
            nmean = stp.tile([1, TC], f32r, name="ln_nmean", tag="ln_nmean")
            with nc.allow_low_precision(reason="f32r LN stats"):
                nc.scalar.activation(out=nmean[:], in_=sum_ps[:], func=AF.Copy,
                                     scale=-1.0 / D)
            ms = stp.tile([1, TC], f32, name="ln_ms", tag="ln_ms")
            nc.scalar.activation(out=ms[:], in_=sumsq_ps[:], func=AF.Copy,
                                 scale=1.0 / D)
            m2 = stp.tile([1, TC], f32, name="ln_m2", tag="ln_m2")
            nc.vector.tensor_mul(out=m2[:], in0=nmean[:], in1=nmean[:])
            var = stp.tile([1, TC], f32, name="ln_var", tag="ln_var")
            nc.vector.tensor_tensor(out=var[:], in0=ms[:], in1=m2[:],
                                    op=ALU.subtract)
            std = stp.tile([1, TC], f32r, name="ln_std", tag="ln_std")
            with nc.allow_low_precision(reason="f32r LN stats"):
                nc.scalar.activation(out=std[:], in_=var[:], func=AF.Sqrt,
                                     bias=eps_t[:], scale=1.0)
            bc_s = ps_bc.tile([P, TC], f32, name="bc_s", tag="bc_a")
            nc.tensor.matmul(out=bc_s[:], lhsT=ones_r[0:1, 0:P], rhs=std[:],
                             start=True, stop=True)
            bc_m = ps_bc.tile([P, TC], f32, name="bc_m", tag="bc_c")
            nc.tensor.matmul(out=bc_m[:], lhsT=ones_r[0:1, 0:P], rhs=nmean[:],
                             start=True, stop=True)
            rstd_bc = sp.tile([P, TC], f32, name="ln_rb", tag="ln_rb")
            nc.vector.reciprocal(out=rstd_bc[:], in_=bc_s[:])
            for j in range(8):
                t1 = sp.tile([P, TC], f32, name="ln_t1", tag="ln_t1")
                nc.vector.tensor_add(out=t1[:], in0=xr[j][:], in1=bc_m[:])
                nc.vector.tensor_mul(out=t1[:], in0=t1[:], in1=rstd_bc[:])
                nc.vector.tensor_scalar(
                    out=out_tiles[j][:], in0=t1[:],
                    scalar1=g_t[:, gcol + j:gcol + j + 1],
                    scalar2=b_t[:, bcol + j:bcol + j + 1],
                    op0=ALU.mult, op1=ALU.add)

        # ================= layers =================
        for l in range(L):
            lnp = sp.tile([P, 48], f32, name="lnp", tag="lnp")
            nc.sync.dma_start(out=lnp[:], in_=lnpp[l])
            b1f_t = sp.tile([P, 32], f32, name="b1f_t", tag="b1f_t")
            nc.sync.dma_start(out=b1f_t[:], in_=b1p[l])

            # ---- LN1 on own tokens -> store -> pairwise AllGather ----
            xln1 = own_tiles()
            layer_norm(lnp, 0, lnp, 8, xln1)
            for j in range(8):
                nc.sync.dma_start(out=xln_locs[l][j * P:(j + 1) * P, :],
                                  in_=xln1[j][:])
            nc.gpsimd.collective_compute(
                "AllGather", ALU.bypass, replica_groups=RG2,
                ins=[xln_locs[l].opt()], outs=[xln_pairs[l].opt()])

            # reload the gathered 1024-token activations (feature-major)
            xf_full = [lp.tile([P, TB], bf16, name=f"xf_{j}", tag=f"xf_{j}")
                       for j in range(8)]
            for k in range(8):
                nc.sync.dma_start(out=xf_full[k][:, 0:TC],
                                  in_=xln_pairs[l][k * P:(k + 1) * P, :])
                nc.sync.dma_start(out=xf_full[k][:, TC:TB],
                                  in_=xln_pairs[l][D + k * P:D + (k + 1) * P, :])

            # ---- layer weight tiles (one contiguous DMA each) ----
            wk_t = wL.tile([P, 8, 4, P], bf16, name="wk_t", tag="wk")
            nc.sync.dma_start(out=wk_t[:], in_=wkp[l])
            wq_t = wL.tile([P, 8, 4, P], bf16, name="wq_t", tag="wq")
            nc.sync.dma_start(out=wq_t[:], in_=wqp[l])
            wv_t = wL.tile([P, 8, 512], bf16, name="wv_t", tag="wv")
            nc.sync.dma_start(out=wv_t[:], in_=wvp[l])
            wo_t = wL.tile([P, 4, 8, P], bf16, name="wo_t", tag="wo")
            nc.sync.dma_start(out=wo_t[:], in_=wop[l])

            # ---- K and Q projections: my 8 heads x 1024 tokens ----
            for dst, wt in ((kq_k, wk_t), (kq_q, wq_t)):
                for oc in range(4):
                    for th in range(2):
                        ps = mmtile()
                        for k in range(8):
                            nc.tensor.matmul(
                                out=ps[:], lhsT=wt[:, k, oc, :],
                                rhs=xf_full[k][:, th * TC:(th + 1) * TC],
                                start=(k == 0), stop=(k == 7))
                        nc.scalar.activation(
                            out=dst[oc][:, th * TC:(th + 1) * TC], in_=ps[:],
                            func=AF.Copy)

            # ---- V projection: [tokens, head-dims], 65-strided with ones ----
            for tcn in range(8):
                ps = mmtile()
                for k in range(8):
                    nc.tensor.matmul(
                        out=ps[:], lhsT=xf_full[k][:, tcn * P:(tcn + 1) * P],
                        rhs=wv_t[:, k, :], start=(k == 0), stop=(k == 7))
                vsb = sp.tile([P, 512], bf16, name="vsb", tag="vsb")
                nc.scalar.activation(out=vsb[:], in_=ps[:], func=AF.Copy)
                nc.sync.dma_start(
                    out=vts[tcn][:].rearrange("p (h c) -> p h c", c=65)[:, :, 0:64],
                    in_=vsb[:].rearrange("p (h c) -> p h c", c=64))

            # ---- attention: 8 local heads, full 1024-token causal span ----
            for hh in range(8):
                kt, qt = kq_k[hh // 2], kq_q[hh // 2]
                base = 64 * (hh % 2)
                ops_lo = ps_o.tile([65, TC], f32, name="ops_lo", tag="ops_lo")
                ops_hi = ps_o.tile([65, TC], f32, name="ops_hi", tag="ops_hi")
                for j in range(8):
                    es_hi = esp.tile([P, TC], bf16, name="es_h", tag=f"es_h{j % 2}")
                    if j < 4:
                        # lo half: q columns [128j, 512), diagonal-masked
                        q0 = P * j
                        sps = mmtile()
                        nc.tensor.matmul(
                            out=sps[:, q0:TC],
                            lhsT=kt[base:base + 64, j * P:(j + 1) * P],
                            rhs=qt[base:base + 64, q0:TC],
                            start=True, stop=True)
                        es_lo = esp.tile([P, TC], bf16, name="es_l",
                                         tag=f"es_l{j % 2}")
                        if j > 0:
                            nc.vector.memset(es_lo[:, 0:q0], 0.0)
                        nc.scalar.activation(out=es_lo[:, q0:TC],
                                             in_=sps[:, q0:TC],
                                             func=AF.Exp, scale=HS ** -0.5)
                        nc.vector.tensor_mul(out=es_lo[:, q0:TC],
                                             in0=es_lo[:, q0:TC],
                                             in1=cm_t[j][:, q0:TC])
                        nc.tensor.matmul(
                            out=ops_lo[:], lhsT=vts[j][:, 65 * hh:65 * hh + 65],
                            rhs=es_lo[:], start=(j == 0), stop=(j == 3))
                        # hi half: q columns [512, 1024), fully visible
                        sps2 = mmtile()
                        nc.tensor.matmul(
                            out=sps2[:],
                            lhsT=kt[base:base + 64, j * P:(j + 1) * P],
                            rhs=qt[base:base + 64, TC:TB],
                            start=True, stop=True)
                        nc.scalar.activation(out=es_hi[:], in_=sps2[:],
                                             func=AF.Exp, scale=HS ** -0.5)
                    else:
                        # hi half only: q columns [512+128(j-4), 1024)
                        q0 = P * (j - 4)
                        sps = mmtile()
                        nc.tensor.matmul(
                            out=sps[:, q0:TC],
                            lhsT=kt[base:base + 64, j * P:(j + 1) * P],
                            rhs=qt[base:base + 64, TC + q0:TB],
                            start=True, stop=True)
                        if j > 4:
                            nc.vector.memset(es_hi[:, 0:q0], 0.0)
                        nc.scalar.activation(out=es_hi[:, q0:TC],
                                             in_=sps[:, q0:TC],
                                             func=AF.Exp, scale=HS ** -0.5)
                        nc.vector.tensor_mul(out=es_hi[:, q0:TC],
                                             in0=es_hi[:, q0:TC],
                                             in1=cm_t[j - 4][:, q0:TC])
                    nc.tensor.matmul(
                        out=ops_hi[:], lhsT=vts[j][:, 65 * hh:65 * hh + 65],
                        rhs=es_hi[:], start=(j == 0), stop=(j == 7))
                # normalize both halves: o = ops[0:64] * exp(-ln(denom))
                for ih, opsx in ((0, ops_lo), (1, ops_hi)):
                    dln = otp.tile([1, TC], f32, name="dln", tag="dln")
                    nc.scalar.activation(out=dln[:], in_=opsx[64:65, :], func=AF.Ln)
                    dr = otp.tile([1, TC], f32r, name="dr", tag="dr")
                    with nc.allow_low_precision(reason="softmax denom recip"):
                        nc.scalar.activation(out=dr[:], in_=dln[:], func=AF.Exp,
                                             scale=-1.0)
                    bc = ps_bc.tile([P, TC], f32, name="bc_o", tag="bc_a")
                    nc.tensor.matmul(out=bc[0:64, :], lhsT=ones_r[0:1, 0:64],
                                     rhs=dr[:], start=True, stop=True)
                    bcs = otp.tile([64, TC], bf16, name="bcs", tag="bcs")
                    nc.scalar.activation(out=bcs[:], in_=bc[0:64, :], func=AF.Copy)
                    nc.vector.tensor_mul(
                        out=o_sb[hh // 2][base:base + 64, ih * TC:(ih + 1) * TC],
                        in0=opsx[0:64, :], in1=bcs[:])

            # ---- Wo partial products -> staging -> pairwise ReduceScatter ----
            for th in range(2):
                for dout in range(8):
                    ps = mmtile()
                    for kc in range(4):
                        nc.tensor.matmul(
                            out=ps[:], lhsT=wo_t[:, kc, dout, :],
                            rhs=o_sb[kc][:, th * TC:(th + 1) * TC],
                            start=(kc == 0), stop=(kc == 3))
                    psb = sp.tile([P, TC], bf16, name="psb", tag="psb")
                    nc.scalar.activation(out=psb[:], in_=ps[:], func=AF.Identity,
                                         bias=lnp[:, 32 + dout:33 + dout], scale=1.0)
                    nc.sync.dma_start(
                        out=part_locs[l][th * D + dout * P:th * D + (dout + 1) * P, :],
                        in_=psb[:])
            nc.gpsimd.collective_compute(
                "ReduceScatter", ALU.add, replica_groups=RG2,
                ins=[part_locs[l].opt()], outs=[attn_rss[l].opt()])

            # ---- residual add from the scattered attention output ----
            for dout in range(8):
                ar = sp.tile([P, TC], bf16, name="ar", tag="ar")
                nc.sync.dma_start(out=ar[:],
                                  in_=attn_rss[l][dout * P:(dout + 1) * P, :])
                nc.vector.tensor_add(out=xr[dout][:], in0=xr[dout][:], in1=ar[:])

            # ---- FFN (own 512 tokens, bf16) ----
            xln2 = own_tiles()
            layer_norm(lnp, 16, lnp, 24, xln2)

            y2 = [big_tile(8 + d_, f"y2_{d_}", dtype=f32) for d_ in range(8)]
            for blk in range(4):
                h1 = [big_tile(c_, f"h1_{c_}") for c_ in range(8)]
                for ci in range(8):
                    hc = 8 * blk + ci
                    wt = wA.tile([P, 8, P], bf16, name="w1_t", tag="wA")
                    nc.sync.dma_start(out=wt[:], in_=w1p[l, hc])
                    ps = mmtile()
                    for k in range(8):
                        nc.tensor.matmul(out=ps[:], lhsT=wt[:, k, :], rhs=xln2[k][:],
                                         start=(k == 0), stop=(k == 7))
                    nc.scalar.activation(out=h1[ci][:], in_=ps[:], func=AF.Relu,
                                         bias=b1f_t[:, hc:hc + 1], scale=1.0)
                for dout in range(8):
                    wt = wA.tile([P, 8, P], bf16, name="w2_t", tag="wA")
                    nc.sync.dma_start(out=wt[:], in_=w2p[l, blk, dout])
                    ps = mmtile()
                    for c in range(8):
                        nc.tensor.matmul(out=ps[:], lhsT=wt[:, c, :], rhs=h1[c][:],
                                         start=(c == 0), stop=(c == 7))
                    if blk == 0:
                        nc.scalar.activation(out=y2[dout][:], in_=ps[:],
                                             func=AF.Identity,
                                             bias=lnp[:, 40 + dout:41 + dout],
                                             scale=1.0)
                    else:
                        nc.vector.tensor_add(out=y2[dout][:], in0=y2[dout][:],
                                             in1=ps[:])
            for dout in range(8):
                nc.vector.tensor_add(out=xr[dout][:], in0=xr[dout][:],
                                     in1=y2[dout][:])

        # ---- final LN (bf16 output for the LM head) ----
        lnf_t = sp.tile([P, 16], f32, name="lnf_t", tag="lnp")
        nc.sync.dma_start(out=lnf_t[:], in_=lnfp[:])
        xlnf = own_tiles()
        layer_norm(lnf_t, 0, lnf_t, 8, xlnf)

        est.close()

        # ================= LM head (local, full vocab) =================
        with tc.tile_pool(name="lmw", bufs=3) as lmw, \
             tc.tile_pool(name="lmo", bufs=3) as lmo, \
             tc.tile_pool(name="ps_lm", bufs=4, space="PSUM") as ps_lm:
            for vb in range(VB):
                wt = lmw.tile([P, 8, P], bf16, name="wout_t", tag="wout")
                nc.sync.dma_start(out=wt[:], in_=woutp[vb])
                ps = ps_lm.tile([P, TC], f32, name="lm_ps", tag="lm")
                for k in range(8):
                    nc.tensor.matmul(out=ps[:], lhsT=wt[:, k, :], rhs=xlnf[k][:],
                                     start=(k == 0), stop=(k == 7))
                osb = lmo.tile([P, TC], f32, name="osb", tag="osb")
                nc.scalar.activation(out=osb[:], in_=ps[:], func=AF.Identity,
                                     bias=bout_t[:, vb:vb + 1], scale=1.0)
                nc.sync.dma_start(out=out[vb * P:(vb + 1) * P, :], in_=osb[:])
        outer.close()

    nc.compile()
    return nc


def _prep_inputs(inputs):
    """Shard/reformat host inputs into 8 per-core input maps."""
    import ml_dtypes
    bf = ml_dtypes.bfloat16
    inp = {k: np.asarray(v) for k, v in inputs.items()}
    tok = inp['input_tokens'].astype(np.int64)          # [B, T]
    temb = np.asarray(inp['tok_emb'], dtype=np.float32)
    pe = np.asarray(inp['pos_emb'], dtype=np.float32)

    w1 = np.asarray(inp['W1'], np.float32)
    w2 = np.asarray(inp['W2'], np.float32)
    shared = {
        'w1p': np.ascontiguousarray(
            w1.reshape(L, 8, P, 32, P).transpose(0, 3, 2, 1, 4)).astype(bf),
        'w2p': np.ascontiguousarray(
            w2.reshape(L, 4, 8, P, 8, P).transpose(0, 1, 4, 3, 2, 5)).astype(bf),
        'woutp': np.ascontiguousarray(
            np.asarray(inp['W_out'], np.float32)
            .reshape(8, P, VB, P).transpose(2, 1, 0, 3)).astype(bf),
        'boutp': np.ascontiguousarray(
            np.asarray(inp['b_out'], np.float32).reshape(VB, P).T),
        'b1p': np.ascontiguousarray(
            np.asarray(inp['b1'], np.float32).reshape(L, 32, P).transpose(0, 2, 1)),
        'lnfp': np.ascontiguousarray(
            np.stack([inp['lnf_g'], inp['lnf_b']], axis=0)
            .reshape(2, 8, P).transpose(2, 0, 1).reshape(P, 16).astype(np.float32)),
    }
    # causal 0/1 masks for diagonal key-chunks (uniform across cores)
    cmask = np.zeros((4, P, TC), np.float32)
    c = np.arange(TC)[None, :]
    p = np.arange(P)[:, None]
    for j in range(4):
        cmask[j] = (c >= P * j + p).astype(np.float32)
    shared['cmask'] = cmask.astype(bf)

    # per-hf weight variants (heads hf*8 .. hf*8+8)
    Wq = np.asarray(inp['Wq'], np.float32)
    Wk = np.asarray(inp['Wk'], np.float32)
    Wv = np.asarray(inp['Wv'], np.float32)
    Wo = np.asarray(inp['Wo'], np.float32)
    hf_w = []
    for hf in range(2):
        hs = slice(hf * 8, hf * 8 + 8)
        WqT = Wq[:, hs].transpose(0, 2, 1, 3).reshape(L, D, 512)
        WkT = Wk[:, hs].transpose(0, 2, 1, 3).reshape(L, D, 512)
        WvT = Wv[:, hs].transpose(0, 2, 1, 3).reshape(L, D, 512)
        wqp = np.ascontiguousarray(
            WqT.reshape(L, 8, P, 4, P).transpose(0, 2, 1, 3, 4)).astype(bf)
        wkp = np.ascontiguousarray(
            WkT.reshape(L, 8, P, 4, P).transpose(0, 2, 1, 3, 4)).astype(bf)
        wvp = np.ascontiguousarray(
            WvT.reshape(L, 8, P, 512).transpose(0, 2, 1, 3)).astype(bf)
        wop = np.ascontiguousarray(
            Wo[:, hf * 512:(hf + 1) * 512, :]
            .reshape(L, 4, P, 8, P).transpose(0, 2, 1, 3, 4)).astype(bf)
        bo = inp['bo'] if hf == 0 else np.zeros_like(inp['bo'])
        lnpp = np.ascontiguousarray(
            np.stack([inp['ln1_g'], inp['ln1_b'], inp['ln2_g'], inp['ln2_b'],
                      bo, inp['b2']], axis=1)
            .reshape(L, 6, 8, P).transpose(0, 3, 1, 2)
            .reshape(L, P, 48).astype(np.float32))
        hf_w.append({'wqp': wqp, 'wkp': wkp, 'wvp': wvp, 'wop': wop,
                     'lnpp': lnpp})

    in_maps = []
    for cix in range(NCORES):
        b, hf = cix // 2, cix % 2
        m = dict(shared)
        m.update(hf_w[hf])
        toks = tok[b, hf * TC:(hf + 1) * TC]
        x0 = temb[toks] + pe[hf * TC:(hf + 1) * TC]      # [TC, D]
        m['x0T'] = np.ascontiguousarray(x0.T, dtype=np.float32)
        in_maps.append(m)
    return in_maps


def _enable_jax_cache():
    try:
        import jax
        jax.config.update("jax_compilation_cache_dir", "/tmp/jax_neff_cache")
        jax.config.update("jax_persistent_cache_min_compile_time_secs", 0.0)
        jax.config.update("jax_persistent_cache_min_entry_size_bytes", -1)
    except Exception:
        pass


def kernel(**inputs):
    global LAST_RESULTS
    _enable_jax_cache()
    from concourse.bass_utils import run_bass_kernel_spmd
    if 'nc' not in _CACHE:
        _CACHE['nc'] = _build()
    nc = _CACHE['nc']
    in_maps = _prep_inputs(inputs)
    kw = {}
    if TRACE:
        kw = dict(trace=True, trace_cores=list(range(NCORES)), stitch_traces=False)
    res = run_bass_kernel_spmd(nc, in_maps, core_ids=list(range(NCORES)), **kw)
    LAST_RESULTS = res
    full = np.empty((B, T, V), np.float32)
    for c in range(NCORES):
        b, hf = c // 2, c % 2
        full[b, hf * TC:(hf + 1) * TC, :] = res.results[c]['out'].T
    return full


# revision 15
# speedup vs baseline: 1.8230x; 1.0021x over previous
"""Self-contained 8-core Trainium2 kernel for the 6-layer dense transformer.

Sharding: batch pairs with head-split attention. Core c owns batch b=c//2 and
sequence half hf=c%2 (512 tokens) for the residual stream, LayerNorms, FFN and
LM head. Attention for batch b is split by heads across the pair: core 2b
computes heads 0-7, core 2b+1 heads 8-15, each over all 1024 tokens, so K/V
never cross cores. The only collectives are pairwise: an AllGather of the LN1
output (so both cores see all 1024 tokens) and a ReduceScatter of the Wo
partial products (each core receives the summed attention output for its own
tokens). The LM head is local: every core holds the full bf16 W_out.

Activations live feature-major (x^T: [D, tokens]); residual stays f32,
LN statistics f32r, everything else (projections, attention, FFN, LM head)
bf16 with f32 PSUM accumulation. Weights are pre-tiled on the host so every
weight DMA is partition-contiguous. Softmax denominators are inverted via
exp(-ln(d)) on the scalar engine (DVE reciprocal is ~25x slower per element).
The embedding lookup runs on the host; the device loads x0^T directly.
"""

import numpy as np

B, T, D, H, HS, L, DFF, V = 4, 1024, 1024, 16, 64, 6, 4096, 32000
NCORES = 8
TC = 512            # tokens owned per core
TB = 1024           # tokens per batch (attention span)
P = 128
VB = V // P         # 250 vocab row-blocks
EPS = 1e-5

_CACHE = {}
TRACE = False
LAST_RESULTS = None


def _build():
    import concourse.bacc as bacc
    import concourse.tile as tile
    import concourse.mybir as mybir
    from contextlib import ExitStack

    f32 = mybir.dt.float32
    f32r = mybir.dt.float32r
    bf16 = mybir.dt.bfloat16
    AF = mybir.ActivationFunctionType
    ALU = mybir.AluOpType

    nc = bacc.Bacc(None, target_bir_lowering=False, debug=False,
                   num_devices=NCORES)

    # ---- parameters (host pre-tiled layouts; wq/wk/wv/wo/lnpp per-core) ----
    x0T = nc.declare_dram_parameter("x0T", [D, TC], f32, isOutput=False)
    wqp = nc.declare_dram_parameter("wqp", [L, P, 8, 4, P], bf16, isOutput=False)
    wkp = nc.declare_dram_parameter("wkp", [L, P, 8, 4, P], bf16, isOutput=False)
    wvp = nc.declare_dram_parameter("wvp", [L, P, 8, 512], bf16, isOutput=False)
    wop = nc.declare_dram_parameter("wop", [L, P, 4, 8, P], bf16, isOutput=False)
    w1p = nc.declare_dram_parameter("w1p", [L, 32, P, 8, P], bf16, isOutput=False)
    w2p = nc.declare_dram_parameter("w2p", [L, 4, 8, P, 8, P], bf16, isOutput=False)
    woutp = nc.declare_dram_parameter("woutp", [VB, P, 8, P], bf16, isOutput=False)
    boutp = nc.declare_dram_parameter("boutp", [P, VB], f32, isOutput=False)
    lnpp = nc.declare_dram_parameter("lnpp", [L, P, 48], f32, isOutput=False)
    b1p = nc.declare_dram_parameter("b1p", [L, P, 32], f32, isOutput=False)
    lnfp = nc.declare_dram_parameter("lnfp", [P, 16], f32, isOutput=False)
    cmask = nc.declare_dram_parameter("cmask", [4, P, TC], bf16, isOutput=False)
    out = nc.declare_dram_parameter("out", [V, TC], f32, isOutput=True)

    RG2 = [[0, 1], [2, 3], [4, 5], [6, 7]]

    with tile.TileContext(nc) as tc:
        outer = ExitStack()
        singles = outer.enter_context(tc.tile_pool(name="singles", bufs=1))
        dramp = outer.enter_context(tc.tile_pool(name="dramp", bufs=1, space="DRAM"))

        # ---- internal DRAM (per layer and split in halves so each collective
        # can start as soon as its half of the data is staged) ----
        HD = D // 2
        xln_locs = [[dramp.tile([HD, TC], bf16, name=f"xln_loc_{i}_{h}")
                     for h in range(2)] for i in range(L)]
        xln_pairs = [[dramp.tile([D, TC], bf16, name=f"xln_pair_{i}_{h}")
                      for h in range(2)] for i in range(L)]
        part_locs = [[dramp.tile([D, TC], bf16, name=f"part_loc_{i}_{h}")
                      for h in range(2)] for i in range(L)]
        attn_rss = [[dramp.tile([HD, TC], bf16, name=f"attn_rs_{i}_{h}")
                     for h in range(2)] for i in range(L)]

        # constants
        ones_f = singles.tile([P, 144], f32, name="ones_f")
        nc.vector.memset(ones_f[:], 1.0)
        ones_r = singles.tile([P, 144], f32r, name="ones_r")
        nc.vector.tensor_copy(out=ones_r[:], in_=ones_f[:])
        eps_t = singles.tile([1, 1], f32, name="eps_t")
        nc.vector.memset(eps_t[:], EPS)
        bout_t = singles.tile([P, VB], f32, name="bout_t")
        nc.sync.dma_start(out=bout_t[:], in_=boutp[:])
        cm_t = []
        for j in range(4):
            mt = singles.tile([P, TC], bf16, name=f"cm_{j}")
            nc.sync.dma_start(out=mt[:], in_=cmask[j])
            cm_t.append(mt)

        est = ExitStack()
        lp = est.enter_context(tc.tile_pool(name="lp", bufs=1))      # xr/xln tiles
        kqp = est.enter_context(tc.tile_pool(name="kqp", bufs=1))    # K/Q [P,1024]
        vtsp = est.enter_context(tc.tile_pool(name="vtsp", bufs=1))  # V 65-strided
        osp = est.enter_context(tc.tile_pool(name="osp", bufs=1))    # o [P,1024]
        big = est.enter_context(tc.tile_pool(name="big", bufs=1))    # h1/y2 slots
        wL = est.enter_context(tc.tile_pool(name="wL", bufs=1))      # layer weights
        wA = est.enter_context(tc.tile_pool(name="wA", bufs=3))      # ffn stream
        sp = est.enter_context(tc.tile_pool(name="sp", bufs=2))      # stream tiles
        esp = est.enter_context(tc.tile_pool(name="esp", bufs=2))    # exp(scores)
        otp = est.enter_context(tc.tile_pool(name="otp", bufs=2))    # denom tmp
        stp = est.enter_context(tc.tile_pool(name="stp", bufs=1))    # LN stats [1,*]

        ps_mm = est.enter_context(tc.tile_pool(name="ps_mm", bufs=2, space="PSUM"))
        ps_o = est.enter_context(tc.tile_pool(name="ps_o", bufs=1, space="PSUM"))
        ps_st = est.enter_context(tc.tile_pool(name="ps_st", bufs=1, space="PSUM"))
        ps_bc = est.enter_context(tc.tile_pool(name="ps_bc", bufs=1, space="PSUM"))

        def mmtile():
            return ps_mm.tile([P, TC], f32, name="mm", tag="mm")

        xr = [lp.tile([P, TC], f32, name=f"xr_{j}", tag=f"xr_{j}") for j in range(8)]

        def own_tiles(dtype=bf16):
            return [lp.tile([P, TC], dtype, name=f"xln_{j}", tag=f"xln_{j}")
                    for j in range(8)]

        # persistent attention tiles. kz: one tile per head, the head's 64 K
        # rows in its parity half and ZEROS in the other half, so the score
        # matmul runs with a full 128x128 stationary (full PE rate); the rhs
        # reads the packed Q tile whose other half contributes 0 via the zeros.
        kz = [kqp.tile([P, TB], bf16, name=f"kz_{i}", tag=f"kz_{i}")
              for i in range(8)]
        for i in range(8):
            nc.vector.memset(kz[i][:], 0.0)
        kq_q = [kqp.tile([P, TB], bf16, name=f"kq_{i}", tag=f"kq_{i}")
                for i in range(4)]
        # 584 wide so a 128-col stationary window starting at 65*hh always fits
        vts = [vtsp.tile([P, 584], bf16, name=f"vts_{i}", tag=f"vts_{i}")
               for i in range(8)]
        for i in range(8):
            nc.vector.memset(vts[i][:], 1.0)   # ones cols (64 of each 65) persist
        o_sb = [osp.tile([P, TB], bf16, name=f"osb_{i}", tag=f"osb_{i}")
                for i in range(4)]

        def big_tile(i, name, dtype=bf16):
            return big.tile([P, TC], dtype, name=name, tag=f"big_{i}")

        # ---- embedding: host-precomputed x0T, straight loads ----
        for j in range(8):
            nc.sync.dma_start(out=xr[j][:], in_=x0T[j * P:(j + 1) * P, :])

        def layer_norm(g_t, gcol, b_t, bcol, out_tiles):
            """xr (f32) -> out_tiles; feature-major LN over partitions."""
            sum_ps = ps_st.tile([1, TC], f32, name="sum_ps", tag="st_a")
            sumsq_ps = ps_st.tile([1, TC], f32, name="sumsq_ps", tag="st_b")
            for j in range(8):
                xc = sp.tile([P, TC], f32r, name="ln_xc", tag="ln_xc")
                nc.scalar.activation(out=xc[:], in_=xr[j][:], func=AF.Copy)
                sq = sp.tile([P, TC], f32r, name="ln_sq", tag="ln_sq")
                nc.scalar.activation(out=sq[:], in_=xr[j][:], func=AF.Square)
                nc.tensor.matmul(out=sum_ps[:], lhsT=ones_r[:, 0:1], rhs=xc[:],
                                 start=(j == 0), stop=(j == 7))
                nc.tensor.matmul(out=sumsq_ps[:], lhsT=ones_r[:, 1:2], rhs=sq[:],
                                 start=(j == 0), stop=(j == 7))
            nmean = stp.tile([1, TC], f32r, name="ln_nmean", tag="ln_nmean")
            with nc.allow_low_precision(reason="f32r LN stats"):
                nc.scalar.activation(out=nmean[:], in_=sum_ps[:], func=AF.Copy,
                                     scale=-1.0 / D)
            ms = stp.tile([1, TC], f32, name="ln_ms", tag="ln_ms")
            nc.scalar.activation(out=ms[:], in_=sumsq_ps[:], func=AF.Copy,
                                 scale=1.0 / D)
            m2 = stp.tile([1, TC], f32, name="ln_m2", tag="ln_m2")
            nc.vector.tensor_mul(out=m2[:], in0=nmean[:], in1=nmean[:])
            var = stp.tile([1, TC], f32, name="ln_var", tag="ln_var")
            nc.vector.tensor_tensor(out=var[:], in0=ms[:], in1=m2[:],
                                    op=ALU.subtract)
            std = stp.tile([1, TC], f32r, name="ln_std", tag="ln_std")
            with nc.allow_low_precision(reason="f32r LN stats"):
                nc.scalar.activation(out=std[:], in_=var[:], func=AF.Sqrt,
                                     bias=eps_t[:], scale=1.0)
            bc_s = ps_bc.tile([P, TC], f32, name="bc_s", tag="bc_a")
            nc.tensor.matmul(out=bc_s[:], lhsT=ones_r[0:1, 0:P], rhs=std[:],
                             start=True, stop=True)
            bc_m = ps_bc.tile([P, TC], f32, name="bc_m", tag="bc_c")
            nc.tensor.matmul(out=bc_m[:], lhsT=ones_r[0:1, 0:P], rhs=nmean[:],
                             start=True, stop=True)
            rstd_bc = sp.tile([P, TC], f32, name="ln_rb", tag="ln_rb")
            nc.vector.reciprocal(out=rstd_bc[:], in_=bc_s[:])
            for j in range(8):
                t1 = sp.tile([P, TC], f32, name="ln_t1", tag="ln_t1")
                nc.vector.tensor_add(out=t1[:], in0=xr[j][:], in1=bc_m[:])
                nc.vector.tensor_mul(out=t1[:], in0=t1[:], in1=rstd_bc[:])
                nc.vector.tensor_scalar(
                    out=out_tiles[j][:], in0=t1[:],
                    scalar1=g_t[:, gcol + j:gcol + j + 1],
                    scalar2=b_t[:, bcol + j:bcol + j + 1],
                    op0=ALU.mult, op1=ALU.add)

        # ================= layers =================
        for l in range(L):
            lnp = sp.tile([P, 48], f32, name="lnp", tag="lnp")
            nc.sync.dma_start(out=lnp[:], in_=lnpp[l])
            b1f_t = sp.tile([P, 32], f32, name="b1f_t", tag="b1f_t")
            nc.sync.dma_start(out=b1f_t[:], in_=b1p[l])

            # ---- LN1 on own tokens -> store -> two pipelined pair AllGathers ----
            xln1 = own_tiles()
            layer_norm(lnp, 0, lnp, 8, xln1)
            for h in range(2):
                for j4 in range(4):
                    nc.sync.dma_start(
                        out=xln_locs[l][h][j4 * P:(j4 + 1) * P, :],
                        in_=xln1[4 * h + j4][:])
                nc.gpsimd.collective_compute(
                    "AllGather", ALU.bypass, replica_groups=RG2,
                    ins=[xln_locs[l][h].opt()], outs=[xln_pairs[l][h].opt()])

            # reload the gathered 1024-token activations (feature-major)
            xf_full = [lp.tile([P, TB], bf16, name=f"xf_{j}", tag=f"xf_{j}")
                       for j in range(8)]
            for k in range(8):
                h, k4 = k // 4, k % 4
                nc.sync.dma_start(out=xf_full[k][:, 0:TC],
                                  in_=xln_pairs[l][h][k4 * P:(k4 + 1) * P, :])
                nc.sync.dma_start(
                    out=xf_full[k][:, TC:TB],
                    in_=xln_pairs[l][h][HD + k4 * P:HD + (k4 + 1) * P, :])

            # ---- layer weight tiles (one contiguous DMA each) ----
            wk_t = wL.tile([P, 8, 4, P], bf16, name="wk_t", tag="wk")
            nc.sync.dma_start(out=wk_t[:], in_=wkp[l])
            wq_t = wL.tile([P, 8, 4, P], bf16, name="wq_t", tag="wq")
            nc.sync.dma_start(out=wq_t[:], in_=wqp[l])
            wv_t = wL.tile([P, 8, 512], bf16, name="wv_t", tag="wv")
            nc.sync.dma_start(out=wv_t[:], in_=wvp[l])
            wo_t = wL.tile([P, 4, 8, P], bf16, name="wo_t", tag="wo")
            nc.sync.dma_start(out=wo_t[:], in_=wop[l])

            # ---- K and Q projections: my 8 heads x 1024 tokens ----
            for oc in range(4):
                for th in range(2):
                    ps = mmtile()
                    for k in range(8):
                        nc.tensor.matmul(
                            out=ps[:], lhsT=wk_t[:, k, oc, :],
                            rhs=xf_full[k][:, th * TC:(th + 1) * TC],
                            start=(k == 0), stop=(k == 7))
                    # split row halves into the two heads' zero-padded K tiles
                    for ph in range(2):
                        nc.scalar.activation(
                            out=kz[2 * oc + ph][64 * ph:64 * ph + 64,
                                                th * TC:(th + 1) * TC],
                            in_=ps[64 * ph:64 * ph + 64, :], func=AF.Copy)
            for oc in range(4):
                for th in range(2):
                    ps = mmtile()
                    for k in range(8):
                        nc.tensor.matmul(
                            out=ps[:], lhsT=wq_t[:, k, oc, :],
                            rhs=xf_full[k][:, th * TC:(th + 1) * TC],
                            start=(k == 0), stop=(k == 7))
                    nc.scalar.activation(
                        out=kq_q[oc][:, th * TC:(th + 1) * TC], in_=ps[:],
                        func=AF.Copy)

            # ---- V projection: [tokens, head-dims], 65-strided with ones ----
            for tcn in range(8):
                ps = mmtile()
                for k in range(8):
                    nc.tensor.matmul(
                        out=ps[:], lhsT=xf_full[k][:, tcn * P:(tcn + 1) * P],
                        rhs=wv_t[:, k, :], start=(k == 0), stop=(k == 7))
                vsb = sp.tile([P, 512], bf16, name="vsb", tag="vsb")
                nc.scalar.activation(out=vsb[:], in_=ps[:], func=AF.Copy)
                nc.sync.dma_start(
                    out=vts[tcn][:, 0:520].rearrange("p (h c) -> p h c",
                                                     c=65)[:, :, 0:64],
                    in_=vsb[:].rearrange("p (h c) -> p h c", c=64))

            # ---- attention: 8 local heads, full 1024-token causal span ----
            for hh in range(8):
                qt = kq_q[hh // 2]
                base = 64 * (hh % 2)
                # alternate PSUM pools by head parity: double-buffered ops
                pso = ps_o if hh % 2 == 0 else ps_st
                tga, tgb = ("ops_lo", "ops_hi") if hh % 2 == 0 else ("st_a", "st_b")
                ops_lo = pso.tile([P, TC], f32, name="ops_lo", tag=tga)
                ops_hi = pso.tile([P, TC], f32, name="ops_hi", tag=tgb)
                vwin = [vts[j][:, 65 * hh:65 * hh + P] for j in range(8)]
                for j in range(8):
                    es_hi = esp.tile([P, TC], bf16, name="es_h", tag=f"es_h{j % 2}")
                    if j < 4:
                        # lo half: q columns [128j, 512), diagonal-masked
                        q0 = P * j
                        sps = mmtile()
                        nc.tensor.matmul(
                            out=sps[:, q0:TC],
                            lhsT=kz[hh][:, j * P:(j + 1) * P],
                            rhs=qt[:, q0:TC],
                            start=True, stop=True)
                        es_lo = esp.tile([P, TC], bf16, name="es_l",
                                         tag=f"es_l{j % 2}")
                        if j > 0:
                            nc.vector.memset(es_lo[:, 0:q0], 0.0)
                        nc.scalar.activation(out=es_lo[:, q0:TC],
                                             in_=sps[:, q0:TC],
                                             func=AF.Exp, scale=HS ** -0.5)
                        nc.vector.tensor_mul(out=es_lo[:, q0:TC],
                                             in0=es_lo[:, q0:TC],
                                             in1=cm_t[j][:, q0:TC])
                        nc.tensor.matmul(
                            out=ops_lo[:], lhsT=vwin[j],
                            rhs=es_lo[:], start=(j == 0), stop=(j == 3))
                        # hi half: q columns [512, 1024), fully visible
                        sps2 = mmtile()
                        nc.tensor.matmul(
                            out=sps2[:],
                            lhsT=kz[hh][:, j * P:(j + 1) * P],
                            rhs=qt[:, TC:TB],
                            start=True, stop=True)
                        nc.scalar.activation(out=es_hi[:], in_=sps2[:],
                                             func=AF.Exp, scale=HS ** -0.5)
                    else:
                        # hi half only: q columns [512+128(j-4), 1024)
                        q0 = P * (j - 4)
                        sps = mmtile()
                        nc.tensor.matmul(
                            out=sps[:, q0:TC],
                            lhsT=kz[hh][:, j * P:(j + 1) * P],
                            rhs=qt[:, TC + q0:TB],
                            start=True, stop=True)
                        if j > 4:
                            nc.vector.memset(es_hi[:, 0:q0], 0.0)
                        nc.scalar.activation(out=es_hi[:, q0:TC],
                                             in_=sps[:, q0:TC],
                                             func=AF.Exp, scale=HS ** -0.5)
                        nc.vector.tensor_mul(out=es_hi[:, q0:TC],
                                             in0=es_hi[:, q0:TC],
                                             in1=cm_t[j - 4][:, q0:TC])
                    nc.tensor.matmul(
                        out=ops_hi[:], lhsT=vwin[j],
                        rhs=es_hi[:], start=(j == 0), stop=(j == 7))
                # normalize both halves: o = ops[0:64] / ops[64] (ones row)
                for ih, opsx in ((0, ops_lo), (1, ops_hi)):
                    dsb = otp.tile([1, TC], f32r, name="dsb", tag="dsb")
                    with nc.allow_low_precision(reason="f32r softmax denom"):
                        nc.scalar.activation(out=dsb[:], in_=opsx[64:65, :],
                                             func=AF.Copy)
                    bc = ps_bc.tile([P, TC], f32, name="bc_o", tag="bc_a")
                    nc.tensor.matmul(out=bc[:], lhsT=ones_r[0:1, 0:P],
                                     rhs=dsb[:], start=True, stop=True)
                    bcr = otp.tile([64, TC], bf16, name="bcr", tag="bcr")
                    with nc.allow_low_precision(reason="bf16 denom recip"):
                        nc.vector.reciprocal(out=bcr[:], in_=bc[0:64, :])
                    nc.vector.tensor_mul(
                        out=o_sb[hh // 2][base:base + 64, ih * TC:(ih + 1) * TC],
                        in0=opsx[0:64, :], in1=bcr[:])

            # ---- Wo partials -> staging -> two pipelined pair ReduceScatters ----
            for h in range(2):
                for d4 in range(4):
                    dout = 4 * h + d4
                    for th in range(2):
                        ps = mmtile()
                        for kc in range(4):
                            nc.tensor.matmul(
                                out=ps[:], lhsT=wo_t[:, kc, dout, :],
                                rhs=o_sb[kc][:, th * TC:(th + 1) * TC],
                                start=(kc == 0), stop=(kc == 3))
                        psb = sp.tile([P, TC], bf16, name="psb", tag="psb")
                        nc.scalar.activation(out=psb[:], in_=ps[:],
                                             func=AF.Identity,
                                             bias=lnp[:, 32 + dout:33 + dout],
                                             scale=1.0)
                        nc.sync.dma_start(
                            out=part_locs[l][h][th * HD + d4 * P:
                                                th * HD + (d4 + 1) * P, :],
                            in_=psb[:])
                nc.gpsimd.collective_compute(
                    "ReduceScatter", ALU.add, replica_groups=RG2,
                    ins=[part_locs[l][h].opt()], outs=[attn_rss[l][h].opt()])

            # ---- residual add from the scattered attention output ----
            for dout in range(8):
                h, d4 = dout // 4, dout % 4
                ar = sp.tile([P, TC], bf16, name="ar", tag="ar")
                nc.sync.dma_start(out=ar[:],
                                  in_=attn_rss[l][h][d4 * P:(d4 + 1) * P, :])
                nc.vector.tensor_add(out=xr[dout][:], in0=xr[dout][:], in1=ar[:])

            # ---- FFN (own 512 tokens, bf16) ----
            xln2 = own_tiles()
            layer_norm(lnp, 16, lnp, 24, xln2)

            y2 = [big_tile(8 + d_, f"y2_{d_}", dtype=f32) for d_ in range(8)]
            for blk in range(4):
                h1 = [big_tile(c_, f"h1_{c_}") for c_ in range(8)]
                for ci in range(8):
                    hc = 8 * blk + ci
                    wt = wA.tile([P, 8, P], bf16, name="w1_t", tag="wA")
                    nc.sync.dma_start(out=wt[:], in_=w1p[l, hc])
                    ps = mmtile()
                    for k in range(8):
                        nc.tensor.matmul(out=ps[:], lhsT=wt[:, k, :], rhs=xln2[k][:],
                                         start=(k == 0), stop=(k == 7))
                    nc.scalar.activation(out=h1[ci][:], in_=ps[:], func=AF.Relu,
                                         bias=b1f_t[:, hc:hc + 1], scale=1.0)
                for dout in range(8):
                    wt = wA.tile([P, 8, P], bf16, name="w2_t", tag="wA")
                    nc.sync.dma_start(out=wt[:], in_=w2p[l, blk, dout])
                    ps = mmtile()
                    for c in range(8):
                        nc.tensor.matmul(out=ps[:], lhsT=wt[:, c, :], rhs=h1[c][:],
                                         start=(c == 0), stop=(c == 7))
                    if blk == 0:
                        nc.scalar.activation(out=y2[dout][:], in_=ps[:],
                                             func=AF.Identity,
                                             bias=lnp[:, 40 + dout:41 + dout],
                                             scale=1.0)
                    else:
                        nc.vector.tensor_add(out=y2[dout][:], in0=y2[dout][:],
                                             in1=ps[:])
            for dout in range(8):
                nc.vector.tensor_add(out=xr[dout][:], in0=xr[dout][:],
                                     in1=y2[dout][:])

        # ---- final LN (bf16 output for the LM head) ----
        lnf_t = sp.tile([P, 16], f32, name="lnf_t", tag="lnp")
        nc.sync.dma_start(out=lnf_t[:], in_=lnfp[:])
        xlnf = own_tiles()
        layer_norm(lnf_t, 0, lnf_t, 8, xlnf)

        est.close()

        # ================= LM head (local, full vocab) =================
        with tc.tile_pool(name="lmw", bufs=3) as lmw, \
             tc.tile_pool(name="lmo", bufs=3) as lmo, \
             tc.tile_pool(name="ps_lm", bufs=4, space="PSUM") as ps_lm:
            for vb in range(VB):
                wt = lmw.tile([P, 8, P], bf16, name="wout_t", tag="wout")
                nc.sync.dma_start(out=wt[:], in_=woutp[vb])
                ps = ps_lm.tile([P, TC], f32, name="lm_ps", tag="lm")
                for k in range(8):
                    nc.tensor.matmul(out=ps[:], lhsT=wt[:, k, :], rhs=xlnf[k][:],
                                     start=(k == 0), stop=(k == 7))
                osb = lmo.tile([P, TC], f32, name="osb", tag="osb")
                nc.scalar.activation(out=osb[:], in_=ps[:], func=AF.Identity,
                                     bias=bout_t[:, vb:vb + 1], scale=1.0)
                nc.sync.dma_start(out=out[vb * P:(vb + 1) * P, :], in_=osb[:])
        outer.close()

    nc.compile()
    return nc


def _prep_inputs(inputs):
    """Shard/reformat host inputs into 8 per-core input maps."""
    import ml_dtypes
    bf = ml_dtypes.bfloat16
    inp = {k: np.asarray(v) for k, v in inputs.items()}
    tok = inp['input_tokens'].astype(np.int64)          # [B, T]
    temb = np.asarray(inp['tok_emb'], dtype=np.float32)
    pe = np.asarray(inp['pos_emb'], dtype=np.float32)

    w1 = np.asarray(inp['W1'], np.float32)
    w2 = np.asarray(inp['W2'], np.float32)
    shared = {
        'w1p': np.ascontiguousarray(
            w1.reshape(L, 8, P, 32, P).transpose(0, 3, 2, 1, 4)).astype(bf),
        'w2p': np.ascontiguousarray(
            w2.reshape(L, 4, 8, P, 8, P).transpose(0, 1, 4, 3, 2, 5)).astype(bf),
        'woutp': np.ascontiguousarray(
            np.asarray(inp['W_out'], np.float32)
            .reshape(8, P, VB, P).transpose(2, 1, 0, 3)).astype(bf),
        'boutp': np.ascontiguousarray(
            np.asarray(inp['b_out'], np.float32).reshape(VB, P).T),
        'b1p': np.ascontiguousarray(
            np.asarray(inp['b1'], np.float32).reshape(L, 32, P).transpose(0, 2, 1)),
        'lnfp': np.ascontiguousarray(
            np.stack([inp['lnf_g'], inp['lnf_b']], axis=0)
            .reshape(2, 8, P).transpose(2, 0, 1).reshape(P, 16).astype(np.float32)),
    }
    # causal 0/1 masks for diagonal key-chunks (uniform across cores)
    cmask = np.zeros((4, P, TC), np.float32)
    c = np.arange(TC)[None, :]
    p = np.arange(P)[:, None]
    for j in range(4):
        cmask[j] = (c >= P * j + p).astype(np.float32)
    shared['cmask'] = cmask.astype(bf)

    # per-hf weight variants (heads hf*8 .. hf*8+8)
    Wq = np.asarray(inp['Wq'], np.float32)
    Wk = np.asarray(inp['Wk'], np.float32)
    Wv = np.asarray(inp['Wv'], np.float32)
    Wo = np.asarray(inp['Wo'], np.float32)
    hf_w = []
    for hf in range(2):
        hs = slice(hf * 8, hf * 8 + 8)
        WqT = Wq[:, hs].transpose(0, 2, 1, 3).reshape(L, D, 512)
        WkT = Wk[:, hs].transpose(0, 2, 1, 3).reshape(L, D, 512)
        WvT = Wv[:, hs].transpose(0, 2, 1, 3).reshape(L, D, 512)
        wqp = np.ascontiguousarray(
            WqT.reshape(L, 8, P, 4, P).transpose(0, 2, 1, 3, 4)).astype(bf)
        wkp = np.ascontiguousarray(
            WkT.reshape(L, 8, P, 4, P).transpose(0, 2, 1, 3, 4)).astype(bf)
        wvp = np.ascontiguousarray(
            WvT.reshape(L, 8, P, 512).transpose(0, 2, 1, 3)).astype(bf)
        wop = np.ascontiguousarray(
            Wo[:, hf * 512:(hf + 1) * 512, :]
            .reshape(L, 4, P, 8, P).transpose(0, 2, 1, 3, 4)).astype(bf)
        bo = inp['bo'] if hf == 0 else np.zeros_like(inp['bo'])
        lnpp = np.ascontiguousarray(
            np.stack([inp['ln1_g'], inp['ln1_b'], inp['ln2_g'], inp['ln2_b'],
                      bo, inp['b2']], axis=1)
            .reshape(L, 6, 8, P).transpose(0, 3, 1, 2)
            .reshape(L, P, 48).astype(np.float32))
        hf_w.append({'wqp': wqp, 'wkp': wkp, 'wvp': wvp, 'wop': wop,
                     'lnpp': lnpp})

    in_maps = []
    for cix in range(NCORES):
        b, hf = cix // 2, cix % 2
        m = dict(shared)
        m.update(hf_w[hf])
        toks = tok[b, hf * TC:(hf + 1) * TC]
        x0 = temb[toks] + pe[hf * TC:(hf + 1) * TC]      # [TC, D]
        m['x0T'] = np.ascontiguousarray(x0.T, dtype=np.float32)
        in_maps.append(m)
    return in_maps


def _enable_jax_cache():
    try:
        import jax
        jax.config.update("jax_compilation_cache_dir", "/tmp/jax_neff_cache")
        jax.config.update("jax_persistent_cache_min_compile_time_secs", 0.0)
        jax.config.update("jax_persistent_cache_min_entry_size_bytes", -1)
    except Exception:
        pass


def kernel(**inputs):
    global LAST_RESULTS
    _enable_jax_cache()
    from concourse.bass_utils import run_bass_kernel_spmd
    if 'nc' not in _CACHE:
        _CACHE['nc'] = _build()
    nc = _CACHE['nc']
    in_maps = _prep_inputs(inputs)
    kw = {}
    if TRACE:
        kw = dict(trace=True, trace_cores=list(range(NCORES)), stitch_traces=False)
    res = run_bass_kernel_spmd(nc, in_maps, core_ids=list(range(NCORES)), **kw)
    LAST_RESULTS = res
    full = np.empty((B, T, V), np.float32)
    for c in range(NCORES):
        b, hf = c // 2, c % 2
        full[b, hf * TC:(hf + 1) * TC, :] = res.results[c]['out'].T
    return full


# revision 19
# speedup vs baseline: 1.9407x; 1.0646x over previous
"""Self-contained 8-core Trainium2 kernel for the 6-layer dense transformer.

Sharding: batch pairs with head-split attention. Core c owns batch b=c//2 and
sequence half hf=c%2 (512 tokens) for the residual stream, LayerNorms, FFN and
LM head. Attention for batch b is split by heads across the pair: core 2b
computes heads 0-7, core 2b+1 heads 8-15, each over all 1024 tokens, so K/V
never cross cores. The only collectives are pairwise: an AllGather of the LN1
output (so both cores see all 1024 tokens) and a ReduceScatter of the Wo
partial products (each core receives the summed attention output for its own
tokens). The LM head is local: every core holds the full bf16 W_out.

Activations live feature-major (x^T: [D, tokens]); residual stays f32,
LN statistics f32r, everything else (projections, attention, FFN, LM head)
bf16 with f32 PSUM accumulation. Weights are pre-tiled on the host so every
weight DMA is partition-contiguous. Softmax denominators are inverted via
exp(-ln(d)) on the scalar engine (DVE reciprocal is ~25x slower per element).
The embedding lookup runs on the host; the device loads x0^T directly.
"""

import numpy as np

B, T, D, H, HS, L, DFF, V = 4, 1024, 1024, 16, 64, 6, 4096, 32000
NCORES = 8
TC = 512            # tokens owned per core
TB = 1024           # tokens per batch (attention span)
P = 128
VB = V // P         # 250 vocab row-blocks
EPS = 1e-5

_CACHE = {}
TRACE = False
LAST_RESULTS = None


def _build():
    import concourse.bacc as bacc
    import concourse.tile as tile
    import concourse.mybir as mybir
    from contextlib import ExitStack

    f32 = mybir.dt.float32
    f32r = mybir.dt.float32r
    bf16 = mybir.dt.bfloat16
    AF = mybir.ActivationFunctionType
    ALU = mybir.AluOpType

    nc = bacc.Bacc(None, target_bir_lowering=False, debug=False,
                   num_devices=NCORES)

    # ---- parameters (host pre-tiled layouts; wq/wk/wv/wo/lnpp per-core) ----
    x0T = nc.declare_dram_parameter("x0T", [D, TC], f32, isOutput=False)
    wqp = nc.declare_dram_parameter("wqp", [L, P, 8, 4, P], bf16, isOutput=False)
    wkp = nc.declare_dram_parameter("wkp", [L, P, 8, 4, P], bf16, isOutput=False)
    wvp = nc.declare_dram_parameter("wvp", [L, P, 8, 512], bf16, isOutput=False)
    wop = nc.declare_dram_parameter("wop", [L, P, 4, 8, P], bf16, isOutput=False)
    w1p = nc.declare_dram_parameter("w1p", [L, 32, P, 8, P], bf16, isOutput=False)
    w2p = nc.declare_dram_parameter("w2p", [L, 4, 8, P, 8, P], bf16, isOutput=False)
    woutp = nc.declare_dram_parameter("woutp", [VB, P, 8, P], bf16, isOutput=False)
    boutp = nc.declare_dram_parameter("boutp", [P, VB], f32, isOutput=False)
    lnpp = nc.declare_dram_parameter("lnpp", [L, P, 48], f32, isOutput=False)
    b1p = nc.declare_dram_parameter("b1p", [L, P, 32], f32, isOutput=False)
    lnfp = nc.declare_dram_parameter("lnfp", [P, 16], f32, isOutput=False)
    cmask = nc.declare_dram_parameter("cmask", [4, P, TC], bf16, isOutput=False)
    out = nc.declare_dram_parameter("out", [V, TC], f32, isOutput=True)

    RG2 = [[0, 1], [2, 3], [4, 5], [6, 7]]

    with tile.TileContext(nc) as tc:
        outer = ExitStack()
        singles = outer.enter_context(tc.tile_pool(name="singles", bufs=1))
        dramp = outer.enter_context(tc.tile_pool(name="dramp", bufs=1, space="DRAM"))

        # ---- internal DRAM (per layer and split in halves so each collective
        # can start as soon as its half of the data is staged) ----
        HD = D // 2
        xln_locs = [[dramp.tile([HD, TC], bf16, name=f"xln_loc_{i}_{h}")
                     for h in range(2)] for i in range(L)]
        xln_pairs = [[dramp.tile([D, TC], bf16, name=f"xln_pair_{i}_{h}")
                      for h in range(2)] for i in range(L)]
        part_locs = [[dramp.tile([D, TC], bf16, name=f"part_loc_{i}_{h}")
                      for h in range(2)] for i in range(L)]
        attn_rss = [[dramp.tile([HD, TC], bf16, name=f"attn_rs_{i}_{h}")
                     for h in range(2)] for i in range(L)]

        # constants
        ones_f = singles.tile([P, 144], f32, name="ones_f")
        nc.vector.memset(ones_f[:], 1.0)
        ones_r = singles.tile([P, 144], f32r, name="ones_r")
        nc.vector.tensor_copy(out=ones_r[:], in_=ones_f[:])
        eps_c = singles.tile([P, 1], f32, name="eps_c")
        nc.vector.memset(eps_c[:], EPS)
        ones_b = singles.tile([P, P], bf16, name="ones_b")
        nc.vector.memset(ones_b[:], 1.0)
        bout_t = singles.tile([P, VB], f32, name="bout_t")
        nc.sync.dma_start(out=bout_t[:], in_=boutp[:])
        cm_t = []
        for j in range(4):
            mt = singles.tile([P, TC], bf16, name=f"cm_{j}")
            nc.sync.dma_start(out=mt[:], in_=cmask[j])
            cm_t.append(mt)

        est = ExitStack()
        lp = est.enter_context(tc.tile_pool(name="lp", bufs=1))      # xr/xln tiles
        kqp = est.enter_context(tc.tile_pool(name="kqp", bufs=1))    # K/Q [P,1024]
        vtsp = est.enter_context(tc.tile_pool(name="vtsp", bufs=1))  # V 65-strided
        osp = est.enter_context(tc.tile_pool(name="osp", bufs=1))    # o [P,1024]
        big = est.enter_context(tc.tile_pool(name="big", bufs=1))    # h1/y2 slots
        wL = est.enter_context(tc.tile_pool(name="wL", bufs=1))      # layer weights
        wA = est.enter_context(tc.tile_pool(name="wA", bufs=3))      # ffn stream
        sp = est.enter_context(tc.tile_pool(name="sp", bufs=2))      # stream tiles
        esp = est.enter_context(tc.tile_pool(name="esp", bufs=3))    # exp(scores)
        otp = est.enter_context(tc.tile_pool(name="otp", bufs=2))    # denom tmp
        stp = est.enter_context(tc.tile_pool(name="stp", bufs=1))    # LN stats [1,*]

        ps_mm = est.enter_context(tc.tile_pool(name="ps_mm", bufs=2, space="PSUM"))
        ps_o = est.enter_context(tc.tile_pool(name="ps_o", bufs=1, space="PSUM"))
        ps_st = est.enter_context(tc.tile_pool(name="ps_st", bufs=1, space="PSUM"))
        ps_bc = est.enter_context(tc.tile_pool(name="ps_bc", bufs=1, space="PSUM"))

        def mmtile():
            return ps_mm.tile([P, TC], f32, name="mm", tag="mm")

        xr = [lp.tile([P, TC], f32, name=f"xr_{j}", tag=f"xr_{j}") for j in range(8)]

        def own_tiles(dtype=bf16):
            return [lp.tile([P, TC], dtype, name=f"xln_{j}", tag=f"xln_{j}")
                    for j in range(8)]

        # persistent attention tiles. kz: one tile per head, the head's 64 K
        # rows in its parity half and ZEROS in the other half, so the score
        # matmul runs with a full 128x128 stationary (full PE rate); the rhs
        # reads the packed Q tile whose other half contributes 0 via the zeros.
        kz = [kqp.tile([P, TB], bf16, name=f"kz_{i}", tag=f"kz_{i}")
              for i in range(8)]
        for i in range(8):
            nc.vector.memset(kz[i][:], 0.0)
        kq_q = [kqp.tile([P, TB], bf16, name=f"kq_{i}", tag=f"kq_{i}")
                for i in range(4)]
        # 584 wide so a 128-col stationary window starting at 65*hh always fits
        vts = [vtsp.tile([P, 584], bf16, name=f"vts_{i}", tag=f"vts_{i}")
               for i in range(8)]
        for i in range(8):
            nc.vector.memset(vts[i][:], 1.0)   # ones cols (64 of each 65) persist
        o_sb = [osp.tile([P, TB], bf16, name=f"osb_{i}", tag=f"osb_{i}")
                for i in range(4)]

        def big_tile(i, name, dtype=bf16):
            return big.tile([P, TC], dtype, name=name, tag=f"big_{i}")

        # ---- embedding: host-precomputed x0T, straight loads ----
        for j in range(8):
            nc.sync.dma_start(out=xr[j][:], in_=x0T[j * P:(j + 1) * P, :])

        def layer_norm(g_t, gcol, b_t, bcol, out_tiles):
            """xr (f32) -> out_tiles; feature-major LN over partitions.

            Stats accumulate through a full 128x128 ones stationary, which
            lands them already broadcast along partitions at full PE rate.
            """
            sum_bc = ps_bc.tile([P, TC], f32, name="sum_bc", tag="bc_a")
            sumsq_bc = ps_bc.tile([P, TC], f32, name="sumsq_bc", tag="bc_c")
            for j in range(8):
                xc = sp.tile([P, TC], bf16, name="ln_xc", tag="ln_xc")
                nc.scalar.activation(out=xc[:], in_=xr[j][:], func=AF.Copy)
                sq = sp.tile([P, TC], bf16, name="ln_sq", tag="ln_sq")
                nc.scalar.activation(out=sq[:], in_=xr[j][:], func=AF.Square)
                nc.tensor.matmul(out=sum_bc[:], lhsT=ones_b[:], rhs=xc[:],
                                 start=(j == 0), stop=(j == 7))
                nc.tensor.matmul(out=sumsq_bc[:], lhsT=ones_b[:], rhs=sq[:],
                                 start=(j == 0), stop=(j == 7))
            nmean = sp.tile([P, TC], f32, name="ln_mb", tag="ln_mb")
            nc.scalar.activation(out=nmean[:], in_=sum_bc[:], func=AF.Copy,
                                 scale=-1.0 / D)
            msq = sp.tile([P, TC], f32, name="ln_msq", tag="ln_msq")
            nc.scalar.activation(out=msq[:], in_=sumsq_bc[:], func=AF.Copy,
                                 scale=1.0 / D)
            m2 = sp.tile([P, TC], f32, name="ln_m2b", tag="ln_m2b")
            nc.vector.tensor_mul(out=m2[:], in0=nmean[:], in1=nmean[:])
            nc.vector.tensor_tensor(out=msq[:], in0=msq[:], in1=m2[:],
                                    op=ALU.subtract)
            std = sp.tile([P, TC], f32, name="ln_sb", tag="ln_sb")
            nc.scalar.activation(out=std[:], in_=msq[:], func=AF.Sqrt,
                                 bias=eps_c[:], scale=1.0)
            rstd_bc = sp.tile([P, TC], f32, name="ln_rb", tag="ln_rb")
            nc.vector.reciprocal(out=rstd_bc[:], in_=std[:])
            for j in range(8):
                t1 = sp.tile([P, TC], f32, name="ln_t1", tag="ln_t1")
                nc.vector.tensor_add(out=t1[:], in0=xr[j][:], in1=nmean[:])
                nc.vector.tensor_mul(out=t1[:], in0=t1[:], in1=rstd_bc[:])
                nc.vector.tensor_scalar(
                    out=out_tiles[j][:], in0=t1[:],
                    scalar1=g_t[:, gcol + j:gcol + j + 1],
                    scalar2=b_t[:, bcol + j:bcol + j + 1],
                    op0=ALU.mult, op1=ALU.add)

        # ================= layers =================
        for l in range(L):
            lnp = sp.tile([P, 48], f32, name="lnp", tag="lnp")
            nc.sync.dma_start(out=lnp[:], in_=lnpp[l])
            b1f_t = sp.tile([P, 32], f32, name="b1f_t", tag="b1f_t")
            nc.sync.dma_start(out=b1f_t[:], in_=b1p[l])

            # ---- LN1 on own tokens -> store -> two pipelined pair AllGathers ----
            xln1 = own_tiles()
            layer_norm(lnp, 0, lnp, 8, xln1)
            for h in range(2):
                for j4 in range(4):
                    nc.sync.dma_start(
                        out=xln_locs[l][h][j4 * P:(j4 + 1) * P, :],
                        in_=xln1[4 * h + j4][:])
                nc.gpsimd.collective_compute(
                    "AllGather", ALU.bypass, replica_groups=RG2,
                    ins=[xln_locs[l][h].opt()], outs=[xln_pairs[l][h].opt()])

            # reload the gathered 1024-token activations (feature-major)
            xf_full = [lp.tile([P, TB], bf16, name=f"xf_{j}", tag=f"xf_{j}")
                       for j in range(8)]
            for k in range(8):
                h, k4 = k // 4, k % 4
                nc.sync.dma_start(out=xf_full[k][:, 0:TC],
                                  in_=xln_pairs[l][h][k4 * P:(k4 + 1) * P, :])
                nc.sync.dma_start(
                    out=xf_full[k][:, TC:TB],
                    in_=xln_pairs[l][h][HD + k4 * P:HD + (k4 + 1) * P, :])

            # ---- layer weight tiles (one contiguous DMA each) ----
            wk_t = wL.tile([P, 8, 4, P], bf16, name="wk_t", tag="wk")
            nc.sync.dma_start(out=wk_t[:], in_=wkp[l])
            wq_t = wL.tile([P, 8, 4, P], bf16, name="wq_t", tag="wq")
            nc.sync.dma_start(out=wq_t[:], in_=wqp[l])
            wv_t = wL.tile([P, 8, 512], bf16, name="wv_t", tag="wv")
            nc.sync.dma_start(out=wv_t[:], in_=wvp[l])
            wo_t = wL.tile([P, 4, 8, P], bf16, name="wo_t", tag="wo")
            nc.sync.dma_start(out=wo_t[:], in_=wop[l])

            # ---- K and Q projections: my 8 heads x 1024 tokens ----
            for oc in range(4):
                for th in range(2):
                    ps = mmtile()
                    for k in range(8):
                        nc.tensor.matmul(
                            out=ps[:], lhsT=wk_t[:, k, oc, :],
                            rhs=xf_full[k][:, th * TC:(th + 1) * TC],
                            start=(k == 0), stop=(k == 7))
                    # split row halves into the two heads' zero-padded K tiles
                    for ph in range(2):
                        nc.scalar.activation(
                            out=kz[2 * oc + ph][64 * ph:64 * ph + 64,
                                                th * TC:(th + 1) * TC],
                            in_=ps[64 * ph:64 * ph + 64, :], func=AF.Copy)
            for oc in range(4):
                for th in range(2):
                    ps = mmtile()
                    for k in range(8):
                        nc.tensor.matmul(
                            out=ps[:], lhsT=wq_t[:, k, oc, :],
                            rhs=xf_full[k][:, th * TC:(th + 1) * TC],
                            start=(k == 0), stop=(k == 7))
                    nc.scalar.activation(
                        out=kq_q[oc][:, th * TC:(th + 1) * TC], in_=ps[:],
                        func=AF.Copy)

            # ---- V projection: [tokens, head-dims], 65-strided with ones ----
            for tcn in range(8):
                ps = mmtile()
                for k in range(8):
                    nc.tensor.matmul(
                        out=ps[:], lhsT=xf_full[k][:, tcn * P:(tcn + 1) * P],
                        rhs=wv_t[:, k, :], start=(k == 0), stop=(k == 7))
                vsb = sp.tile([P, 512], bf16, name="vsb", tag="vsb")
                nc.scalar.activation(out=vsb[:], in_=ps[:], func=AF.Copy)
                nc.sync.dma_start(
                    out=vts[tcn][:, 0:520].rearrange("p (h c) -> p h c",
                                                     c=65)[:, :, 0:64],
                    in_=vsb[:].rearrange("p (h c) -> p h c", c=64))

            # ---- attention: 8 local heads, interleaved in pairs so one head's
            # matmuls fill the other's softmax bubbles ----
            for hp in range(4):
                qt = kq_q[hp]
                opst = [None, None]
                for sl in range(2):       # slot: even head -> ps_o, odd -> ps_st
                    pso = ps_o if sl == 0 else ps_st
                    tga, tgb = (("ops_lo", "ops_hi") if sl == 0
                                else ("st_a", "st_b"))
                    opst[sl] = (pso.tile([P, TC], f32, name="ops_lo", tag=tga),
                                pso.tile([P, TC], f32, name="ops_hi", tag=tgb))
                for j in range(8):
                    for sl in range(2):
                        hh = 2 * hp + sl
                        ops_lo, ops_hi = opst[sl]
                        vwin = vts[j][:, 65 * hh:65 * hh + P]
                        es_hi = esp.tile([P, TC], bf16, name="es_h",
                                         tag=f"es_h{sl}")
                        if j < 4:
                            # lo half: q columns [128j, 512), diagonal-masked
                            q0 = P * j
                            sps = mmtile()
                            nc.tensor.matmul(
                                out=sps[:, q0:TC],
                                lhsT=kz[hh][:, j * P:(j + 1) * P],
                                rhs=qt[:, q0:TC],
                                start=True, stop=True)
                            es_lo = esp.tile([P, TC], bf16, name="es_l",
                                             tag=f"es_l{sl}")
                            if j > 0:
                                nc.vector.memset(es_lo[:, 0:q0], 0.0)
                            nc.scalar.activation(out=es_lo[:, q0:TC],
                                                 in_=sps[:, q0:TC],
                                                 func=AF.Exp, scale=HS ** -0.5)
                            nc.vector.tensor_mul(out=es_lo[:, q0:TC],
                                                 in0=es_lo[:, q0:TC],
                                                 in1=cm_t[j][:, q0:TC])
                            nc.tensor.matmul(
                                out=ops_lo[:], lhsT=vwin,
                                rhs=es_lo[:], start=(j == 0), stop=(j == 3))
                            # hi half: q columns [512, 1024), fully visible
                            sps2 = mmtile()
                            nc.tensor.matmul(
                                out=sps2[:],
                                lhsT=kz[hh][:, j * P:(j + 1) * P],
                                rhs=qt[:, TC:TB],
                                start=True, stop=True)
                            nc.scalar.activation(out=es_hi[:], in_=sps2[:],
                                                 func=AF.Exp, scale=HS ** -0.5)
                        else:
                            # hi half only: q columns [512+128(j-4), 1024)
                            q0 = P * (j - 4)
                            sps = mmtile()
                            nc.tensor.matmul(
                                out=sps[:, q0:TC],
                                lhsT=kz[hh][:, j * P:(j + 1) * P],
                                rhs=qt[:, TC + q0:TB],
                                start=True, stop=True)
                            if j > 4:
                                nc.vector.memset(es_hi[:, 0:q0], 0.0)
                            nc.scalar.activation(out=es_hi[:, q0:TC],
                                                 in_=sps[:, q0:TC],
                                                 func=AF.Exp, scale=HS ** -0.5)
                            nc.vector.tensor_mul(out=es_hi[:, q0:TC],
                                                 in0=es_hi[:, q0:TC],
                                                 in1=cm_t[j - 4][:, q0:TC])
                        nc.tensor.matmul(
                            out=ops_hi[:], lhsT=vwin,
                            rhs=es_hi[:], start=(j == 0), stop=(j == 7))
                # normalize: o = ops[0:64] / ops[64] (ones row)
                for sl in range(2):
                    hh = 2 * hp + sl
                    base = 64 * sl
                    for ih, opsx in ((0, opst[sl][0]), (1, opst[sl][1])):
                        dsb = otp.tile([1, TC], f32r, name="dsb", tag="dsb")
                        with nc.allow_low_precision(reason="f32r softmax denom"):
                            nc.scalar.activation(out=dsb[:], in_=opsx[64:65, :],
                                                 func=AF.Copy)
                        bc = ps_bc.tile([P, TC], f32, name="bc_o", tag="bc_a")
                        nc.tensor.matmul(out=bc[:], lhsT=ones_r[0:1, 0:P],
                                         rhs=dsb[:], start=True, stop=True)
                        bcr = otp.tile([64, TC], bf16, name="bcr", tag="bcr")
                        with nc.allow_low_precision(reason="bf16 denom recip"):
                            nc.vector.reciprocal(out=bcr[:], in_=bc[0:64, :])
                        nc.vector.tensor_mul(
                            out=o_sb[hp][base:base + 64, ih * TC:(ih + 1) * TC],
                            in0=opsx[0:64, :], in1=bcr[:])

            # ---- Wo partials -> staging -> two pipelined pair ReduceScatters ----
            for h in range(2):
                for d4 in range(4):
                    dout = 4 * h + d4
                    for th in range(2):
                        ps = mmtile()
                        for kc in range(4):
                            nc.tensor.matmul(
                                out=ps[:], lhsT=wo_t[:, kc, dout, :],
                                rhs=o_sb[kc][:, th * TC:(th + 1) * TC],
                                start=(kc == 0), stop=(kc == 3))
                        psb = sp.tile([P, TC], bf16, name="psb", tag="psb")
                        nc.scalar.activation(out=psb[:], in_=ps[:],
                                             func=AF.Identity,
                                             bias=lnp[:, 32 + dout:33 + dout],
                                             scale=1.0)
                        nc.sync.dma_start(
                            out=part_locs[l][h][th * HD + d4 * P:
                                                th * HD + (d4 + 1) * P, :],
                            in_=psb[:])
                nc.gpsimd.collective_compute(
                    "ReduceScatter", ALU.add, replica_groups=RG2,
                    ins=[part_locs[l][h].opt()], outs=[attn_rss[l][h].opt()])

            # ---- residual add from the scattered attention output ----
            for dout in range(8):
                h, d4 = dout // 4, dout % 4
                ar = sp.tile([P, TC], bf16, name="ar", tag="ar")
                nc.sync.dma_start(out=ar[:],
                                  in_=attn_rss[l][h][d4 * P:(d4 + 1) * P, :])
                nc.vector.tensor_add(out=xr[dout][:], in0=xr[dout][:], in1=ar[:])

            # ---- FFN (own 512 tokens, bf16) ----
            xln2 = own_tiles()
            layer_norm(lnp, 16, lnp, 24, xln2)

            y2 = [big_tile(8 + d_, f"y2_{d_}", dtype=f32) for d_ in range(8)]
            for blk in range(4):
                h1 = [big_tile(c_, f"h1_{c_}") for c_ in range(8)]
                for ci in range(8):
                    hc = 8 * blk + ci
                    wt = wA.tile([P, 8, P], bf16, name="w1_t", tag="wA")
                    nc.sync.dma_start(out=wt[:], in_=w1p[l, hc])
                    ps = mmtile()
                    for k in range(8):
                        nc.tensor.matmul(out=ps[:], lhsT=wt[:, k, :], rhs=xln2[k][:],
                                         start=(k == 0), stop=(k == 7))
                    nc.scalar.activation(out=h1[ci][:], in_=ps[:], func=AF.Relu,
                                         bias=b1f_t[:, hc:hc + 1], scale=1.0)
                for dout in range(8):
                    wt = wA.tile([P, 8, P], bf16, name="w2_t", tag="wA")
                    nc.sync.dma_start(out=wt[:], in_=w2p[l, blk, dout])
                    ps = mmtile()
                    for c in range(8):
                        nc.tensor.matmul(out=ps[:], lhsT=wt[:, c, :], rhs=h1[c][:],
                                         start=(c == 0), stop=(c == 7))
                    if blk == 0:
                        nc.scalar.activation(out=y2[dout][:], in_=ps[:],
                                             func=AF.Identity,
                                             bias=lnp[:, 40 + dout:41 + dout],
                                             scale=1.0)
                    else:
                        nc.vector.tensor_add(out=y2[dout][:], in0=y2[dout][:],
                                             in1=ps[:])
            for dout in range(8):
                nc.vector.tensor_add(out=xr[dout][:], in0=xr[dout][:],
                                     in1=y2[dout][:])

        # ---- final LN (bf16 output for the LM head) ----
        lnf_t = sp.tile([P, 16], f32, name="lnf_t", tag="lnp")
        nc.sync.dma_start(out=lnf_t[:], in_=lnfp[:])
        xlnf = own_tiles()
        layer_norm(lnf_t, 0, lnf_t, 8, xlnf)

        est.close()

        # ================= LM head (local, full vocab) =================
        with tc.tile_pool(name="lmw", bufs=3) as lmw, \
             tc.tile_pool(name="lmo", bufs=3) as lmo, \
             tc.tile_pool(name="ps_lm", bufs=4, space="PSUM") as ps_lm:
            for vb in range(VB):
                wt = lmw.tile([P, 8, P], bf16, name="wout_t", tag="wout")
                nc.sync.dma_start(out=wt[:], in_=woutp[vb])
                ps = ps_lm.tile([P, TC], f32, name="lm_ps", tag="lm")
                for k in range(8):
                    nc.tensor.matmul(out=ps[:], lhsT=wt[:, k, :], rhs=xlnf[k][:],
                                     start=(k == 0), stop=(k == 7))
                osb = lmo.tile([P, TC], f32, name="osb", tag="osb")
                nc.scalar.activation(out=osb[:], in_=ps[:], func=AF.Identity,
                                     bias=bout_t[:, vb:vb + 1], scale=1.0)
                nc.sync.dma_start(out=out[vb * P:(vb + 1) * P, :], in_=osb[:])
        outer.close()

    nc.compile()
    return nc


def _prep_inputs(inputs):
    """Shard/reformat host inputs into 8 per-core input maps."""
    import ml_dtypes
    bf = ml_dtypes.bfloat16
    inp = {k: np.asarray(v) for k, v in inputs.items()}
    tok = inp['input_tokens'].astype(np.int64)          # [B, T]
    temb = np.asarray(inp['tok_emb'], dtype=np.float32)
    pe = np.asarray(inp['pos_emb'], dtype=np.float32)

    w1 = np.asarray(inp['W1'], np.float32)
    w2 = np.asarray(inp['W2'], np.float32)
    shared = {
        'w1p': np.ascontiguousarray(
            w1.reshape(L, 8, P, 32, P).transpose(0, 3, 2, 1, 4)).astype(bf),
        'w2p': np.ascontiguousarray(
            w2.reshape(L, 4, 8, P, 8, P).transpose(0, 1, 4, 3, 2, 5)).astype(bf),
        'woutp': np.ascontiguousarray(
            np.asarray(inp['W_out'], np.float32)
            .reshape(8, P, VB, P).transpose(2, 1, 0, 3)).astype(bf),
        'boutp': np.ascontiguousarray(
            np.asarray(inp['b_out'], np.float32).reshape(VB, P).T),
        'b1p': np.ascontiguousarray(
            np.asarray(inp['b1'], np.float32).reshape(L, 32, P).transpose(0, 2, 1)),
        'lnfp': np.ascontiguousarray(
            np.stack([inp['lnf_g'], inp['lnf_b']], axis=0)
            .reshape(2, 8, P).transpose(2, 0, 1).reshape(P, 16).astype(np.float32)),
    }
    # causal 0/1 masks for diagonal key-chunks (uniform across cores)
    cmask = np.zeros((4, P, TC), np.float32)
    c = np.arange(TC)[None, :]
    p = np.arange(P)[:, None]
    for j in range(4):
        cmask[j] = (c >= P * j + p).astype(np.float32)
    shared['cmask'] = cmask.astype(bf)

    # per-hf weight variants (heads hf*8 .. hf*8+8)
    Wq = np.asarray(inp['Wq'], np.float32)
    Wk = np.asarray(inp['Wk'], np.float32)
    Wv = np.asarray(inp['Wv'], np.float32)
    Wo = np.asarray(inp['Wo'], np.float32)
    hf_w = []
    for hf in range(2):
        hs = slice(hf * 8, hf * 8 + 8)
        WqT = Wq[:, hs].transpose(0, 2, 1, 3).reshape(L, D, 512)
        WkT = Wk[:, hs].transpose(0, 2, 1, 3).reshape(L, D, 512)
        WvT = Wv[:, hs].transpose(0, 2, 1, 3).reshape(L, D, 512)
        wqp = np.ascontiguousarray(
            WqT.reshape(L, 8, P, 4, P).transpose(0, 2, 1, 3, 4)).astype(bf)
        wkp = np.ascontiguousarray(
            WkT.reshape(L, 8, P, 4, P).transpose(0, 2, 1, 3, 4)).astype(bf)
        wvp = np.ascontiguousarray(
            WvT.reshape(L, 8, P, 512).transpose(0, 2, 1, 3)).astype(bf)
        wop = np.ascontiguousarray(
            Wo[:, hf * 512:(hf + 1) * 512, :]
            .reshape(L, 4, P, 8, P).transpose(0, 2, 1, 3, 4)).astype(bf)
        bo = inp['bo'] if hf == 0 else np.zeros_like(inp['bo'])
        lnpp = np.ascontiguousarray(
            np.stack([inp['ln1_g'], inp['ln1_b'], inp['ln2_g'], inp['ln2_b'],
                      bo, inp['b2']], axis=1)
            .reshape(L, 6, 8, P).transpose(0, 3, 1, 2)
            .reshape(L, P, 48).astype(np.float32))
        hf_w.append({'wqp': wqp, 'wkp': wkp, 'wvp': wvp, 'wop': wop,
                     'lnpp': lnpp})

    in_maps = []
    for cix in range(NCORES):
        b, hf = cix // 2, cix % 2
        m = dict(shared)
        m.update(hf_w[hf])
        toks = tok[b, hf * TC:(hf + 1) * TC]
        x0 = temb[toks] + pe[hf * TC:(hf + 1) * TC]      # [TC, D]
        m['x0T'] = np.ascontiguousarray(x0.T, dtype=np.float32)
        in_maps.append(m)
    return in_maps


def _enable_jax_cache():
    try:
        import jax
        jax.config.update("jax_compilation_cache_dir", "/tmp/jax_neff_cache")
        jax.config.update("jax_persistent_cache_min_compile_time_secs", 0.0)
        jax.config.update("jax_persistent_cache_min_entry_size_bytes", -1)
    except Exception:
        pass


def kernel(**inputs):
    global LAST_RESULTS
    _enable_jax_cache()
    from concourse.bass_utils import run_bass_kernel_spmd
    if 'nc' not in _CACHE:
        _CACHE['nc'] = _build()
    nc = _CACHE['nc']
    in_maps = _prep_inputs(inputs)
    kw = {}
    if TRACE:
        kw = dict(trace=True, trace_cores=list(range(NCORES)), stitch_traces=False)
    res = run_bass_kernel_spmd(nc, in_maps, core_ids=list(range(NCORES)), **kw)
    LAST_RESULTS = res
    full = np.empty((B, T, V), np.float32)
    for c in range(NCORES):
        b, hf = c // 2, c % 2
        full[b, hf * TC:(hf + 1) * TC, :] = res.results[c]['out'].T
    return full
